# revision 70
# baseline (speedup 1.0000x reference)
"""Trainium2 Bass kernel for nn_CDEM_62079457296798 (channel-attention
transformer block).

Sharding: 8 cores = 4 batches x 2 spatial halves (64 rows + 1 halo row each).
Cross-core communication: two small per-band-pair AllReduces carrying the
channel-attention Gram matrices and q/k l2-norm sums; everything else local.

Layout: channel-major activations [C_part, pixels_free]; attention channels
padded 48 -> 64 per head. Heavy use of fp8e4m3 DoubleRow matmuls (2 K-planes
per instruction; planes interleaved in memory so the PE streams 2 rows/cycle):
the q/kv 1x1 convs pair the 192 input channels as [96, 2]; the depthwise 3x3
runs as 4 tap-pair DoubleRow matmuls with diagonal [128, 2, 128] weights
(overlapping-stride pair APs over the padded image) + 1 bf16-free center tap;
ffn1/ffn2 pair K the same way. q/k sq-norms ride the Gram matmuls
(qg = q^T [q|k], kg = k^T k; diag extracted via masked reduce). z and lin are
fused: W_comb = beta*lin*attn is built once after softmax, so the per-chunk
trunk is W_comb^T v -> t1 -> ffn -> proj (proj in bf16). All runtime scalars
(alpha/beta/gamma/delta) are folded host-side; kernel-side rescales are
compile-time powers of two. The trunk is software-pipelined (tp one chunk
ahead) and interleaved with v-band production to keep the PE stream dense.
"""
import sys
sys.path.insert(0, '/opt/trn_rl_repo')

import numpy as np
import ml_dtypes

import bass_rust
from concourse import bacc, mybir, tile
from concourse.bass import _add_dep_helper
from concourse.bass_utils import run_bass_kernel_spmd

F32 = mybir.dt.float32
F32R = mybir.dt.float32r
BF16 = mybir.dt.bfloat16
FP8 = mybir.dt.float8e4
DRM = mybir.MatmulPerfMode.DoubleRow
AF = mybir.ActivationFunctionType
OP = mybir.AluOpType
bf16 = ml_dtypes.bfloat16
f8 = ml_dtypes.float8_e4m3fn

# depthwise 3x3 as 4 fp8 DoubleRow pairs + 1 single (tap index t = 3*(dr+1)+(dc+1));
# pair strides in elements of the [ER, EC] image (2 = two cols, 260 = two rows)
DW_PAIRS = [(0, 2, 2), (3, 5, 2), (6, 8, 2), (1, 7, 2 * 130)]
DW_SINGLE = 4


def _pair_ap(base, stride):
    raw = base.ap.copy()
    return bass_rust.AP(base.tensor, base.offset,
                        [raw[0], [stride, 2]] + list(raw[1:]))

N_CORES = 8
B, C, H, W = 4, 192, 128, 128
HEADS, CH = 4, 48
CPH = 64                # padded channels per head
CP = HEADS * CPH        # 256 padded attn channels
HLOC = 64               # image rows per core
ER, EC = 66, 130        # ext rows/cols (halo + zero pad)
NEXT = ER * EC          # 8580
NLOC = HLOC * W         # 8192
NCK = 16                # output chunks (4 rows x 128 = 512 px)
CONV_CHUNKS = [(i * 512, 512) for i in range(16)] + [(16 * 512, NEXT - 16 * 512)]
GRP = 2048
CONV_GROUPS = [(i * GRP, GRP) for i in range(4)] + [(4 * GRP, NEXT - 4 * GRP)]
KB = [(0, 128), (128, 64)]          # 192-channel K bands

DIRECT_PSUM_OUT = False  # DMA cannot read PSUM on TRN2


import os
STAGE = int(os.environ.get("KSTAGE", "4"))
KSUB = int(os.environ.get("KSUB", "4"))


class _StageDone(Exception):
    pass


def build_nc():
    nc = bacc.Bacc("TRN2", target_bir_lowering=False, debug=False,
                   num_devices=N_CORES)

    d_xe = nc.dram_tensor("xe", [96, NEXT, 2], FP8, kind="ExternalInput")
    d_ye = nc.dram_tensor("ye", [96, NEXT, 2], FP8, kind="ExternalInput")
    d_yc = nc.dram_tensor("yc", [C, NLOC], BF16, kind="ExternalInput")
    d_wq = nc.dram_tensor("wq", [96, 2, CP], FP8, kind="ExternalInput")
    d_wkv = nc.dram_tensor("wkv", [96, 2, 2 * CP], FP8, kind="ExternalInput")
    d_qdw = nc.dram_tensor("qdw", [CP, 9, 128], FP8, kind="ExternalInput")
    d_kvdw = nc.dram_tensor("kvdw", [2 * CP, 9, 128], FP8, kind="ExternalInput")
    d_wlin = nc.dram_tensor("wlin", [128, 2, C], BF16, kind="ExternalInput")
    d_wf1 = nc.dram_tensor("wf1", [96, 2, 768], FP8, kind="ExternalInput")
    d_wf2 = nc.dram_tensor("wf2", [128, 3, 2, C], FP8, kind="ExternalInput")
    d_wpr = nc.dram_tensor("wpr", [96, 2, C], BF16, kind="ExternalInput")
    d_tempb = nc.dram_tensor("tempb", [128, 2], F32, kind="ExternalInput")
    d_gamma = nc.dram_tensor("gamma", [128, 1], F32, kind="ExternalInput")
    d_id128 = nc.dram_tensor("id128", [128, 128], F32, kind="ExternalInput")
    d_out = nc.dram_tensor("out", [C, NLOC], F32, kind="ExternalOutput")
    cc_in = [nc.dram_tensor(f"cc_in{p}", [112, 114], F32) for p in range(2)]
    cc_out = [nc.dram_tensor(f"cc_out{p}", [112, 114], F32) for p in range(2)]

    with tile.TileContext(nc) as tc:
        with (
            tc.tile_pool(name="sbw", bufs=1) as sbw,      # weights/consts
            tc.tile_pool(name="sbpre", bufs=2) as sbpre,  # conv1x1 out (ext img)
            tc.tile_pool(name="sbin", bufs=3) as sbin,    # streamed conv inputs
            tc.tile_pool(name="sbqk", bufs=4) as sbqk,    # q/k chunk tiles
            tc.tile_pool(name="sbT", bufs=1) as sbT,      # qT/kT/v persistents
            tc.tile_pool(name="sbs", bufs=1) as sbs,      # small attn tiles
            tc.tile_pool(name="sbc", bufs=3) as sbc,      # trunk chunk pipeline
            tc.tile_pool(name="sbg", bufs=6) as sbg,      # gelu chunk tiles
            tc.tile_pool(name="pcv", bufs=3, space="PSUM") as pcv,
            tc.tile_pool(name="pdw", bufs=2, space="PSUM") as pdw,
            tc.tile_pool(name="psm", bufs=1, space="PSUM") as psm,
        ):
            # ---------- weights ----------
            wq_t = sbw.tile([96, 2, CP], FP8, tag="wq", name="wq")
            wkv_t = sbw.tile([96, 2, 2 * CP], FP8, tag="wkv", name="wkv")
            nc.sync.dma_start(wq_t[:], d_wq.ap())
            qdw_t = [sbw.tile([128, 9, 128], FP8, tag=f"qdw{m}", name=f"qdw{m}") for m in range(2)]
            kvdw_t = [sbw.tile([128, 9, 128], FP8, tag=f"kvdw{m}", name=f"kvdw{m}") for m in range(4)]
            id128 = sbw.tile([128, 128], F32, tag="id128", name="id128")
            for m in range(2):
                nc.sync.dma_start(qdw_t[m][:], d_qdw[128 * m:128 * (m + 1)])
            nc.sync.dma_start(id128[:], d_id128.ap())

            def load_kv_weights():
                nc.sync.dma_start(wkv_t[:], d_wkv.ap())
                for m in range(4):
                    nc.sync.dma_start(kvdw_t[m][:], d_kvdw[128 * m:128 * (m + 1)])
            wlin_t = sbw.tile([128, 2, C], BF16, tag="wlin", name="wlin")
            wf1_t = sbw.tile([96, 2, 768], FP8, tag="wf1", name="wf1")
            wf2_t = sbw.tile([128, 3, 2, C], FP8, tag="wf2", name="wf2")
            wpr_t = sbw.tile([96, 2, C], BF16, tag="wpr", name="wpr")
            tempb = sbw.tile([128, 2], F32, tag="tempb", name="tempb")
            gscv = sbw.tile([128, 1], F32, tag="gscv", name="gscv")

            def load_trunk_weights():
                nc.sync.dma_start(wlin_t[:], d_wlin.ap())
                nc.sync.dma_start(wf1_t[:], d_wf1.ap())
                nc.sync.dma_start(wf2_t[:], d_wf2.ap())
                nc.sync.dma_start(wpr_t[:], d_wpr.ap())
                nc.sync.dma_start(tempb[:], d_tempb.ap())
                nc.sync.dma_start(gscv[:], d_gamma.ap())

            # persistent attn-path results; qkT packs q (cols 0:112) and
            # k (cols 112:224) transposed per band-pair
            qkT = [sbT.tile([128, 64, 224], BF16, tag=f"qkT{p}", name=f"qkT{p}")
                   for p in range(2)]
            vband = [sbT.tile([128, NLOC], BF16, tag=f"v{m}", name=f"v{m}") for m in range(2)]
            # gram + norm accumulators live in the tp-tag PSUM (idle pre-trunk)
            qgacc = pcv.tile([112, 448], F32, tag="tp", bufs=2, name="qgacc")
            kgacc = pcv.tile([112, 224], F32, tag="tp", bufs=2, name="kgacc")

            # ============ q/k/v production ============
            def band_producer(src_dram, w_t, dw_tiles, m, sink,
                              collect_mms=None):
                """One 128-wide band: conv1x1 (fp8 DR) + depthwise 3x3.
                Returns (need_groups, emit_dw) for interleaved emission."""
                pre = sbpre.tile([128, ER, EC], FP8, tag="pre", name="pre")
                pref = pre[:].rearrange("p a b -> p (a b)")
                state = {"g": 0, "ci": 0}

                def need_groups(ng):
                    while state["g"] < min(ng, len(CONV_GROUPS)):
                        g0, gn = CONV_GROUPS[state["g"]]
                        xc = sbin.tile([96, GRP, 2], FP8, tag="xin", name="xin")
                        nc.sync.dma_start(xc[:, :gn, :],
                                          src_dram[:, g0:g0 + gn, :])
                        for c0 in range(0, gn, 512):
                            cn = min(512, gn - c0)
                            ps = pcv.tile([128, 512], F32, tag="cv", name="cv")
                            mm = nc.tensor.matmul(
                                ps[:, :cn],
                                w_t[:, :, 128 * m:128 * (m + 1)],
                                xc[:, c0:c0 + cn, :].rearrange(
                                    "p n two -> p two n"),
                                start=True, stop=True, perf_mode=DRM)
                            if collect_mms is not None:
                                collect_mms.append(mm)
                            if state["ci"] % 2 == 0:
                                nc.vector.tensor_copy(
                                    pref[:, g0 + c0:g0 + c0 + cn], ps[:, :cn])
                            else:
                                nc.scalar.copy(
                                    pref[:, g0 + c0:g0 + c0 + cn], ps[:, :cn])
                            state["ci"] += 1
                        state["g"] += 1

                def emit_dw(ck):
                    r0 = 1 + 4 * ck
                    dp = pdw.tile([128, 4, 128], F32, tag="dw", name="dw")
                    for i, (ta, tb, stride) in enumerate(DW_PAIRS):
                        dra, dca = ta // 3 - 1, ta % 3 - 1
                        base = pre[:, r0 + dra:r0 + 4 + dra,
                                   1 + dca:129 + dca]
                        nc.tensor.matmul(
                            dp[:], dw_tiles[m][:, 2 * i:2 * i + 2, :],
                            _pair_ap(base, stride),
                            start=(i == 0), stop=False, perf_mode=DRM)
                    nc.tensor.matmul(
                        dp[:], dw_tiles[m][:, 8, :],
                        pre[:, r0:r0 + 4, 1:129],
                        start=False, stop=True)
                    sink(m, ck, dp[:].rearrange("p a b -> p (a b)"))

                return need_groups, emit_dw

            def conv_dw_path(src_dram, w_t, dw_tiles, n_mb, sink, m_off=0,
                             collect_mms=None):
                for m in range(m_off, m_off + n_mb):
                    ng, edw = band_producer(src_dram, w_t, dw_tiles, m, sink,
                                            collect_mms)
                    ng(len(CONV_GROUPS))
                    for ck in range(NCK if KSUB >= 2 else 0):
                        edw(ck)

            def qk_sink(coff):
                qcbig = [None]

                def sink(m, ck, flat):
                    j = ck % 4
                    if j == 0:
                        qcbig[0] = sbqk.tile([128, 2048], BF16, tag="qkc", name="qkc")
                    qc = qcbig[0][:, 512 * j:512 * (j + 1)]
                    if ck % 2 == 0:
                        nc.vector.tensor_copy(qc, flat)
                    else:
                        nc.scalar.copy(qc, flat)
                    if KSUB >= 4 and j == 3:
                        nc.sync.dma_start_transpose(
                            qkT[m][:, 4 * ck - 12:4 * ck + 4, coff:coff + 112],
                            qcbig[0][0:112, :])
                return sink

            def v_sink(m, ck, flat):
                dst = vband[m - 2]
                if ck % 2 == 0:
                    nc.vector.tensor_copy(dst[:, ck * 512:(ck + 1) * 512], flat)
                else:
                    nc.scalar.copy(dst[:, ck * 512:(ck + 1) * 512], flat)

            sinkq = qk_sink(0)
            sinkk = qk_sink(112)
            sqv = sbs.tile([128, 2], F32, tag="sqv", name="sqv")
            skv = sbs.tile([128, 2], F32, tag="skv", name="skv")

            def run_band(src, w_t, dwt, m, sink):
                ng, edw = band_producer(src, w_t, dwt, m, sink)
                ng(len(CONV_GROUPS))
                for ck in range(NCK if KSUB >= 2 else 0):
                    edw(ck)

            def gram_chunks(p, ck0, ck1):
                # gram + q/k sq-norms for band-pair p (qg: [q^T q | q^T k],
                # kg: k^T k)
                for ck in range(ck0, ck1):
                    nc.tensor.matmul(qgacc[:, 224 * p:224 * (p + 1)],
                                     qkT[p][:, ck, 0:112], qkT[p][:, ck, :],
                                     start=(ck == 0), stop=(ck == 63))
                    nc.tensor.matmul(kgacc[:, 112 * p:112 * (p + 1)],
                                     qkT[p][:, ck, 112:224],
                                     qkT[p][:, ck, 112:224],
                                     start=(ck == 0), stop=(ck == 63))

            def finish_ar(p):
                dsc = sbs.tile([112, 112], F32, tag="dsc", name="dsc")
                nc.vector.tensor_tensor(
                    out=dsc[:], in0=qgacc[:, 224 * p:224 * p + 112],
                    in1=id128[0:112, 0:112], op=OP.mult)
                nc.vector.tensor_reduce(sqv[0:112, p:p + 1], dsc[:],
                                        axis=mybir.AxisListType.X, op=OP.add)
                dsc2 = sbs.tile([112, 112], F32, tag="dsc2", name="dsc2")
                nc.vector.tensor_tensor(
                    out=dsc2[:], in0=kgacc[:, 112 * p:112 * (p + 1)],
                    in1=id128[0:112, 0:112], op=OP.mult)
                nc.vector.tensor_reduce(skv[0:112, p:p + 1], dsc2[:],
                                        axis=mybir.AxisListType.X, op=OP.add)
                gsb = sbs.tile([112, 114], F32, tag=f"gsb{p}", name=f"gsb{p}")
                nc.vector.tensor_copy(gsb[:, 0:112],
                                      qgacc[:, 224 * p + 112:224 * (p + 1)])
                nc.scalar.dma_start(cc_in[p].ap()[:, 0:112], gsb[:, 0:112])
                nc.scalar.dma_start(cc_in[p].ap()[:, 112:113], sqv[0:112, p:p + 1])
                nc.scalar.dma_start(cc_in[p].ap()[:, 113:114], skv[0:112, p:p + 1])
                nc.gpsimd.collective_compute(
                    "AllReduce", OP.add,
                    replica_groups=[[0, 1], [2, 3], [4, 5], [6, 7]],
                    ins=[cc_in[p].ap()], outs=[cc_out[p].ap()])

            run_band(d_xe, wq_t, qdw_t, 0, sinkq)
            load_kv_weights()
            run_band(d_ye, wkv_t, kvdw_t, 0, sinkk)
            run_band(d_xe, wq_t, qdw_t, 1, sinkq)
            if STAGE >= 3:
                gram_chunks(0, 0, 64)
                finish_ar(0)
            run_band(d_ye, wkv_t, kvdw_t, 1, sinkk)
            load_trunk_weights()
            if STAGE >= 3:
                gram_chunks(1, 0, 64)
                finish_ar(1)

            if STAGE < 3:
                oc0 = sbs.tile([128, 2], F32, tag="oc0d", name="oc0d")
                nc.vector.tensor_copy(oc0[:], sqv[:])
                nc.sync.dma_start(d_out[0:128, 0:2], oc0[:])
            if STAGE >= 3:
                # v(m2) + v(m3) head start overlap the AllReduces
                v_mms = []
                conv_dw_path(d_ye, wkv_t, kvdw_t, 1, v_sink, m_off=2,
                             collect_mms=v_mms)
                vng3, vdw3 = band_producer(d_ye, wkv_t, kvdw_t, 3, v_sink,
                                           collect_mms=v_mms)
                vng3(len(CONV_GROUPS))
                for ck in range(6):
                    vdw3(ck)

                gg = sbs.tile([112, 224], F32, tag="gg", name="gg")
                sqg = sbs.tile([128, 2], F32, tag="sqg", name="sqg")
                skg = sbs.tile([128, 2], F32, tag="skg", name="skg")
                nc.vector.memset(sqg[:], 1.0)
                nc.vector.memset(skg[:], 1.0)
                for p in range(2):
                    nc.sync.dma_start(gg[:, 112 * p:112 * (p + 1)],
                                      cc_out[p].ap()[:, 0:112])
                    nc.sync.dma_start(sqg[0:112, p:p + 1],
                                      cc_out[p].ap()[:, 112:113])
                    nc.sync.dma_start(skg[0:112, p:p + 1],
                                      cc_out[p].ap()[:, 113:114])

                # ============ attention finalize ============
                def rsqrt_newton(tag, s_t):
                    sc = sbs.tile([128, 2], F32, tag=tag + "_c")
                    nc.vector.tensor_scalar_max(sc[:], s_t[:], 1e-24)
                    rt = sbs.tile([128, 2], F32, tag=tag + "_s")
                    nc.scalar.activation(rt[:], sc[:], AF.Sqrt)
                    r0 = sbs.tile([128, 2], F32, tag=tag + "_r0")
                    nc.vector.reciprocal(r0[:], rt[:])
                    rr = sbs.tile([128, 2], F32, tag=tag + "_rr")
                    nc.vector.tensor_tensor(out=rr[:], in0=r0[:], in1=r0[:], op=OP.mult)
                    t1_ = sbs.tile([128, 2], F32, tag=tag + "_t1")
                    nc.vector.scalar_tensor_tensor(out=t1_[:], in0=sc[:], scalar=-0.5,
                                                   in1=rr[:], op0=OP.mult, op1=OP.mult)
                    nc.vector.tensor_scalar_add(t1_[:], t1_[:], 1.5)
                    rv = sbs.tile([128, 2], F32, tag=tag)
                    nc.vector.tensor_tensor(out=rv[:], in0=r0[:], in1=t1_[:], op=OP.mult)
                    return rv

                rq = rsqrt_newton("rq", sqg)
                rk = rsqrt_newton("rk", skg)
                srow = sbs.tile([128, 2], F32, tag="srow", name="srow")
                nc.vector.tensor_tensor(out=srow[:], in0=rq[:], in1=tempb[:], op=OP.mult)

                srow_r, scol_r = [], []
                for p in range(2):
                    for src, lst, nm in ((srow, srow_r, "sr"), (rk, scol_r, "sc")):
                        fp = psm.tile([1, 112], F32, tag="sm", name="sm")
                        nc.tensor.transpose(fp[:], src[0:112, p:p + 1],
                                            id128[0:112, 0:112])
                        fr = sbs.tile([1, 112], F32R, tag=f"{nm}{p}", name=f"{nm}{p}")
                        nc.vector.tensor_copy(fr[:], fp[:])
                        lst.append(fr)

                # W_comb = beta*lin*attn, [v-ch(pad 128), plane p, out 192];
                # pad v rows stay zero (vband pad rows are zero anyway)
                wcs = sbs.tile([128, 2, C], BF16, tag="wcs", name="wcs")
                nc.gpsimd.memset(wcs[:], 0.0)
                for p in range(2):
                    spair = psm.tile([112, 112], F32, tag="sm", name="sm")
                    nc.tensor.matmul(spair[:], srow_r[p][:], scol_r[p][:],
                                     start=True, stop=True)
                    lg = sbs.tile([112, 112], F32, tag="lg", name="lg")
                    nc.vector.tensor_tensor(out=lg[:], in0=gg[:, 112 * p:112 * (p + 1)],
                                            in1=spair[:], op=OP.mult)
                    at16 = sbs.tile([112, 112], BF16, tag="at16", name="at16")
                    for e in range(2):
                        sl = slice(64 * e, 64 * e + 48)
                        mx = sbs.tile([112, 1], F32, tag="mx", name="mx")
                        nc.vector.tensor_reduce(mx[sl, :], lg[sl, sl],
                                                axis=mybir.AxisListType.X, op=OP.max)
                        exh = sbs.tile([112, 112], F32, tag="exh", name="exh")
                        nc.vector.tensor_scalar(out=exh[sl, 0:48], in0=lg[sl, sl],
                                                scalar1=mx[sl, :], scalar2=None,
                                                op0=OP.subtract)
                        ex2 = sbs.tile([112, 112], F32, tag="ex2", name="ex2")
                        den = sbs.tile([112, 1], F32, tag="den", name="den")
                        nc.scalar.activation(ex2[sl, 0:48], exh[sl, 0:48], AF.Exp,
                                             accum_out=den[sl, :])
                        rc0 = sbs.tile([112, 1], F32, tag="rc0", name="rc0")
                        nc.vector.reciprocal(rc0[sl, :], den[sl, :])
                        nt = sbs.tile([112, 1], F32, tag="nt", name="nt")
                        nc.vector.tensor_tensor(out=nt[sl, :], in0=den[sl, :],
                                                in1=rc0[sl, :], op=OP.mult)
                        nc.vector.tensor_scalar(out=nt[sl, :], in0=nt[sl, :],
                                                scalar1=-1.0, scalar2=2.0,
                                                op0=OP.mult, op1=OP.add)
                        rc1 = sbs.tile([112, 1], F32, tag="rc1", name="rc1")
                        nc.vector.tensor_tensor(out=rc1[sl, :], in0=rc0[sl, :],
                                                in1=nt[sl, :], op=OP.mult)
                        nc.vector.tensor_scalar(out=at16[sl, 0:48], in0=ex2[sl, 0:48],
                                                scalar1=rc1[sl, :], scalar2=None,
                                                op0=OP.mult)
                        wcp = pcv.tile([128, 512], F32, tag="cv", name="cv")
                        nc.tensor.matmul(wcp[sl, :C], at16[sl, 0:48],
                                         wlin_t[64 * e:64 * e + 48, p, :],
                                         start=True, stop=True)
                        nc.vector.tensor_copy(wcs[sl, p, :], wcp[sl, :C])

                if STAGE < 4:
                    for ck in range(6, NCK):
                        vdw3(ck)
                    ocx = sbs.tile([112, 64], F32, tag="ocx", name="ocx")
                    nc.vector.tensor_copy(ocx[:], wcs[0:112, 0, 0:64])
                    nc.sync.dma_start(d_out[0:112, 0:64], ocx[:])
                # ==== software-pipelined trunk, interleaved with v band-3 ====
                # stage A(ck): v-dw chunk (6 ahead) + tp matmuls — one iter ahead
                # stage B(ck): t1 -> ffn1 -> gelu -> ffn2 -> t2 -> proj
                tp_q, t1_q = {}, {}

                def stage_a(ck):
                    if ck + 6 < NCK:
                        vdw3(ck + 6)
                    c0 = ck * 512
                    # fused z+lin: tp = W_comb^T v = 256x t'_true
                    tp = [pcv.tile([128, 512], F32, tag="tp", bufs=2,
                                   name="tp") for _ in range(2)]
                    for mi in range(2):
                        for p in range(2):
                            nc.tensor.matmul(tp[mi][:96, :],
                                             wcs[:, p, 96 * mi:96 * (mi + 1)],
                                             vband[p][:, c0:c0 + 512],
                                             start=(p == 0), stop=(p == 1))
                    tp_q[ck] = tp

                def stage_t1(ck):
                    tp = tp_q.pop(ck)
                    c0 = ck * 512
                    ycn = sbc.tile([96, 2, 512], BF16, tag="ycn", name="ycn")
                    for mi in range(2):
                        nc.sync.dma_start(ycn[:, mi, :],
                                          d_yc[96 * mi:96 * (mi + 1), c0:c0 + 512])
                    # t1f = gamma*t1 = ycn(= g*a*y) + tp * (gamma/256)
                    t1f = sbc.tile([96, 2, 512], BF16, tag="t1f", name="t1f")
                    for mi in range(2):
                        nc.vector.scalar_tensor_tensor(
                            out=t1f[:, mi, :], in0=tp[mi][:96, :],
                            scalar=gscv[0:96, :],
                            in1=ycn[:, mi, :], op0=OP.mult, op1=OP.add)
                    t1c8 = sbc.tile([96, 512, 2], FP8, tag="t1c8", name="t1c8")
                    nc.vector.tensor_copy(
                        t1c8[:].rearrange("p n two -> p two n"), t1f[:])
                    t1_q[ck] = (t1f, t1c8)

                def stage_b(ck):
                    t1f, t1c8 = t1_q.pop(ck)
                    c0 = ck * 512
                    # ffn1 + gelu: fp1 = 8x f1_true; gc8 = gelu(f1_true)
                    gc8 = [sbg.tile([128, 512, 2], FP8, tag="gc8", name="gc8")
                           for _ in range(3)]
                    for mt in range(6):
                        fp1 = pcv.tile([128, 512], F32, tag="cv", name="cv")
                        nc.tensor.matmul(fp1[:],
                                         wf1_t[:, :, 128 * mt:128 * (mt + 1)],
                                         t1c8[:].rearrange("p n two -> p two n"),
                                         start=True, stop=True,
                                         perf_mode=DRM)
                        nc.scalar.activation(gc8[mt // 2][:, :, mt % 2], fp1[:],
                                             AF.Gelu, scale=0.125)
                    # ffn2: fp2 = 8*delta*f2_true; t2 = t1f + fp2/8
                    t2c = sbc.tile([96, 2, 512], BF16, tag="t2c", name="t2c")
                    for mi in range(2):
                        fp2 = pcv.tile([128, 512], F32, tag="cv", name="cv")
                        for jp in range(3):
                            nc.tensor.matmul(fp2[:96, :],
                                             wf2_t[:, jp, :, 96 * mi:96 * (mi + 1)],
                                             gc8[jp][:].rearrange(
                                                 "p n two -> p two n"),
                                             start=(jp == 0),
                                             stop=(jp == 2), perf_mode=DRM)
                        nc.vector.scalar_tensor_tensor(
                            out=t2c[:, mi, :], in0=fp2[:96, :], scalar=0.125,
                            in1=t1f[:, mi, :], op0=OP.mult, op1=OP.add)
                    # proj (bf16): accumulate over the two 96-ch planes
                    for mi, (mo, ms) in enumerate(KB):
                        pp = pcv.tile([128, 512], F32, tag="cv", name="cv")
                        for pl in range(2):
                            nc.tensor.matmul(pp[:ms, :],
                                             wpr_t[:, pl, mo:mo + ms],
                                             t2c[:, pl, :],
                                             start=(pl == 0), stop=(pl == 1))
                        oc = sbc.tile([128, 512], F32, tag=f"oc{mi}", name=f"oc{mi}")
                        if mi == 0:
                            nc.scalar.copy(oc[:ms, :], pp[:ms, :])
                        else:
                            nc.vector.tensor_copy(oc[:ms, :], pp[:ms, :])
                        nc.sync.dma_start(d_out[mo:mo + ms, c0:c0 + 512],
                                          oc[:ms, :])

                if STAGE >= 4:
                    stage_a(0)
                    stage_t1(0)
                    for ck in range(NCK):
                        if ck + 1 < NCK:
                            stage_a(ck + 1)
                        stage_b(ck)
                        if ck + 1 < NCK:
                            stage_t1(ck + 1)

    nc.compile()
    return nc


_NC = None


def _get_nc():
    global _NC
    if _NC is None:
        _NC = build_nc()
    return _NC


def _prep_weights(q_w, q_dw_w, kv_w, kv_dw_w, linear_w, proj_w, ffn1_w, ffn2_w,
                  temperature, alpha, beta, gamma, delta):
    def pad_oc(w):  # [192 real oc, ic] -> [ic, 256 padded oc]
        out = np.zeros((C, CP), np.float32)
        for h in range(HEADS):
            out[:, CPH * h:CPH * h + CH] = w[CH * h:CH * (h + 1), :].T
        return out

    wq = pad_oc(np.asarray(q_w, np.float32)) * 8.0
    kv = np.asarray(kv_w, np.float32)
    wkv = np.concatenate([pad_oc(kv[:C]), pad_oc(kv[C:])], axis=1) * 8.0

    # [192,1,3,3] -> [256, 9, 128] diag, slots = DW_PAIRS order + center
    slot_tap = [0, 2, 3, 5, 6, 8, 1, 7, 4]

    def pad_dw(w):
        out = np.zeros((CP, 9, 128), np.float32)
        for h in range(HEADS):
            for j in range(CH):
                cp = CPH * h + j
                taps = w[CH * h + j, 0].reshape(9)
                for s, t in enumerate(slot_tap):
                    out[cp, s, cp % 128] = taps[t]
        return out * 32.0

    qdw = pad_dw(np.asarray(q_dw_w, np.float32))
    kvd = np.asarray(kv_dw_w, np.float32)
    kvdw = np.concatenate([pad_dw(kvd[:C]), pad_dw(kvd[C:])], axis=0)

    gamma_f = float(gamma)
    # wlin (bf16) = beta*lin padded; W_comb = attn x wlin -> tp = 256x t'_true
    lin = np.asarray(linear_w, np.float32) * float(beta)
    wlin = np.zeros((CP, C), np.float32)
    for h in range(HEADS):
        wlin[CPH * h:CPH * h + CH, :] = lin[:, CH * h:CH * (h + 1)].T
    wlin8 = wlin.reshape(2, 128, C).transpose(1, 0, 2)

    # t1c8 = gamma*t1; wf18 = ffn1_w^T * 8/gamma -> fp1 = 8x f1_true
    wf1 = np.asarray(ffn1_w, np.float32).T * (8.0 / gamma_f)
    wf18 = wf1.reshape(2, 96, 768).transpose(1, 0, 2)
    # wf28 = ffn2_w^T * 8*delta -> fp2 = 8*delta*f2_true
    wf2 = np.asarray(ffn2_w, np.float32).T * (8.0 * float(delta))
    wf28 = wf2.reshape(3, 2, 128, C).transpose(2, 0, 1, 3)
    wpr = np.asarray(proj_w, np.float32).T.reshape(2, 96, C).transpose(1, 0, 2)

    tempb = np.zeros((128, 2), np.float32)
    tv = np.asarray(temperature, np.float32).reshape(HEADS)
    for h in range(HEADS):
        tempb[64 * (h % 2):64 * (h % 2) + 64, h // 2] = tv[h]

    gscv = np.full((128, 1), gamma_f / 256.0, np.float32)
    id128 = np.eye(128, dtype=np.float32)

    return {
        "_yscale": gamma_f * float(alpha),
        "wq": wq.reshape(2, 96, CP).transpose(1, 0, 2).astype(f8),
        "wkv": wkv.reshape(2, 96, 2 * CP).transpose(1, 0, 2).astype(f8),
        "qdw": qdw.astype(f8), "kvdw": kvdw.astype(f8),
        "wlin": wlin8.astype(bf16).copy(), "wf1": wf18.astype(f8),
        "wf2": wf28.astype(f8), "wpr": wpr.astype(bf16).copy(),
        "tempb": tempb, "gamma": gscv,
        "id128": id128,
    }


def _make_in_maps(x, y, shared):
    shared = dict(shared)
    yscale = shared.pop("_yscale")
    in_maps = []
    for c in range(N_CORES):
        bi, s = c // 2, c % 2
        r0 = s * HLOC
        xe = np.zeros((C, ER, EC), np.float32)
        ye = np.zeros((C, ER, EC), np.float32)
        rlo, rhi = max(r0 - 1, 0), min(r0 + HLOC + 1, H)
        elo = rlo - (r0 - 1)
        xe[:, elo:elo + (rhi - rlo), 1:129] = x[bi, :, rlo:rhi, :]
        ye[:, elo:elo + (rhi - rlo), 1:129] = y[bi, :, rlo:rhi, :]
        m = dict(shared)
        m["xe"] = xe.reshape(2, 96, NEXT).transpose(1, 2, 0).astype(f8)
        m["ye"] = ye.reshape(2, 96, NEXT).transpose(1, 2, 0).astype(f8)
        m["yc"] = (yscale * y[bi, :, r0:r0 + HLOC, :]
                   ).reshape(C, NLOC).astype(bf16)
        in_maps.append(m)
    return in_maps


def kernel(**inputs):
    x = np.asarray(inputs["x"], np.float32)
    y = np.asarray(inputs["y"], np.float32)
    shared = _prep_weights(
        inputs["q_w"], inputs["q_dw_w"], inputs["kv_w"], inputs["kv_dw_w"],
        inputs["linear_w"], inputs["proj_w"], inputs["ffn1_w"], inputs["ffn2_w"],
        inputs["temperature"], inputs["alpha"], inputs["beta"],
        inputs["gamma"], inputs["delta"])

    in_maps = _make_in_maps(x, y, shared)

    nc = _get_nc()
    res = run_bass_kernel_spmd(nc, in_maps, list(range(N_CORES)))
    out = np.empty((B, C, H, W), np.float32)
    for c in range(N_CORES):
        bi, s = c // 2, c % 2
        out[bi, :, s * HLOC:(s + 1) * HLOC, :] = \
            res.results[c]["out"].reshape(C, HLOC, W)
    return out



# revision 71
# speedup vs baseline: 1.1236x; 1.1236x over previous
"""Trainium2 Bass kernel for nn_CDEM_62079457296798 (channel-attention
transformer block).

Sharding: 8 cores = 4 batches x 2 spatial halves (64 rows + 1 halo row each).
Cross-core communication: two small per-band-pair AllReduces carrying the
channel-attention Gram matrices and q/k l2-norm sums; everything else local.

Layout: channel-major activations [C_part, pixels_free]; attention channels
padded 48 -> 64 per head. Heavy use of fp8e4m3 DoubleRow matmuls (2 K-planes
per instruction; planes interleaved in memory so the PE streams 2 rows/cycle):
the q/kv 1x1 convs pair the 192 input channels as [96, 2]; the depthwise 3x3
runs as 4 tap-pair DoubleRow matmuls with diagonal [128, 2, 128] weights
(overlapping-stride pair APs over the padded image) + 1 bf16-free center tap;
ffn1/ffn2 pair K the same way. q/k sq-norms ride the Gram matmuls
(qg = q^T [q|k], kg = k^T k; diag extracted via masked reduce). z and lin are
fused: W_comb = beta*lin*attn is built once after softmax, so the per-chunk
trunk is W_comb^T v -> t1 -> ffn -> proj (proj in bf16). All runtime scalars
(alpha/beta/gamma/delta) are folded host-side; kernel-side rescales are
compile-time powers of two. The trunk is software-pipelined (tp one chunk
ahead) and interleaved with v-band production to keep the PE stream dense.
"""
import sys
sys.path.insert(0, '/opt/trn_rl_repo')

import numpy as np
import ml_dtypes

import bass_rust
from concourse import bacc, mybir, tile
from concourse.bass import _add_dep_helper
from concourse.bass_utils import run_bass_kernel_spmd

F32 = mybir.dt.float32
F32R = mybir.dt.float32r
BF16 = mybir.dt.bfloat16
FP8 = mybir.dt.float8e4
DRM = mybir.MatmulPerfMode.DoubleRow
AF = mybir.ActivationFunctionType
OP = mybir.AluOpType
bf16 = ml_dtypes.bfloat16
f8 = ml_dtypes.float8_e4m3fn

# depthwise 3x3 as 4 fp8 DoubleRow pairs + 1 single (tap index t = 3*(dr+1)+(dc+1));
# pair strides in elements of the [ER, EC] image (2 = two cols, 260 = two rows)
DW_PAIRS = [(0, 2, 2), (3, 5, 2), (6, 8, 2), (1, 7, 2 * 130)]
DW_SINGLE = 4


def _pair_ap(base, stride):
    raw = base.ap.copy()
    return bass_rust.AP(base.tensor, base.offset,
                        [raw[0], [stride, 2]] + list(raw[1:]))

N_CORES = 8
B, C, H, W = 4, 192, 128, 128
HEADS, CH = 4, 48
CPH = 64                # padded channels per head
CP = HEADS * CPH        # 256 padded attn channels
HLOC = 64               # image rows per core
ER, EC = 66, 130        # ext rows/cols (halo + zero pad)
NEXT = ER * EC          # 8580
NLOC = HLOC * W         # 8192
NCK = 16                # output chunks (4 rows x 128 = 512 px)
CONV_CHUNKS = [(i * 512, 512) for i in range(16)] + [(16 * 512, NEXT - 16 * 512)]
GRP = 2048
CONV_GROUPS = [(i * GRP, GRP) for i in range(4)] + [(4 * GRP, NEXT - 4 * GRP)]
KB = [(0, 128), (128, 64)]          # 192-channel K bands

DIRECT_PSUM_OUT = False  # DMA cannot read PSUM on TRN2


import os
STAGE = int(os.environ.get("KSTAGE", "4"))
KSUB = int(os.environ.get("KSUB", "4"))


class _StageDone(Exception):
    pass


def build_nc():
    nc = bacc.Bacc("TRN2", target_bir_lowering=False, debug=False,
                   num_devices=N_CORES)

    d_xe = nc.dram_tensor("xe", [96, NEXT, 2], FP8, kind="ExternalInput")
    d_ye = nc.dram_tensor("ye", [96, NEXT, 2], FP8, kind="ExternalInput")
    d_yc = nc.dram_tensor("yc", [C, NLOC], BF16, kind="ExternalInput")
    d_wq = nc.dram_tensor("wq", [96, 2, CP], FP8, kind="ExternalInput")
    d_wkv = nc.dram_tensor("wkv", [96, 2, 2 * CP], FP8, kind="ExternalInput")
    d_qdw = nc.dram_tensor("qdw", [CP, 9, 128], FP8, kind="ExternalInput")
    d_kvdw = nc.dram_tensor("kvdw", [2 * CP, 9, 128], FP8, kind="ExternalInput")
    d_wlin = nc.dram_tensor("wlin", [128, 2, C], BF16, kind="ExternalInput")
    d_wf1 = nc.dram_tensor("wf1", [96, 2, 768], FP8, kind="ExternalInput")
    d_wf2 = nc.dram_tensor("wf2", [128, 3, 2, C], FP8, kind="ExternalInput")
    d_wpr = nc.dram_tensor("wpr", [96, 2, C], BF16, kind="ExternalInput")
    d_tempb = nc.dram_tensor("tempb", [128, 2], F32, kind="ExternalInput")
    d_gamma = nc.dram_tensor("gamma", [128, 1], F32, kind="ExternalInput")
    d_id128 = nc.dram_tensor("id128", [128, 128], F32, kind="ExternalInput")
    d_out = nc.dram_tensor("out", [C, NLOC], F32, kind="ExternalOutput")
    cc_in = [nc.dram_tensor(f"cc_in{p}", [112, 114], F32) for p in range(2)]
    cc_out = [nc.dram_tensor(f"cc_out{p}", [112, 114], F32) for p in range(2)]

    with tile.TileContext(nc) as tc:
        with (
            tc.tile_pool(name="sbw", bufs=1) as sbw,      # weights/consts
            tc.tile_pool(name="sbpre", bufs=2) as sbpre,  # conv1x1 out (ext img)
            tc.tile_pool(name="sbin", bufs=3) as sbin,    # streamed conv inputs
            tc.tile_pool(name="sbqk", bufs=4) as sbqk,    # q/k chunk tiles
            tc.tile_pool(name="sbT", bufs=1) as sbT,      # qT/kT/v persistents
            tc.tile_pool(name="sbs", bufs=1) as sbs,      # small attn tiles
            tc.tile_pool(name="sbc", bufs=3) as sbc,      # trunk chunk pipeline
            tc.tile_pool(name="sbg", bufs=6) as sbg,      # gelu chunk tiles
            tc.tile_pool(name="pcv", bufs=3, space="PSUM") as pcv,
            tc.tile_pool(name="pdw", bufs=2, space="PSUM") as pdw,
            tc.tile_pool(name="psm", bufs=1, space="PSUM") as psm,
        ):
            # ---------- weights ----------
            wq_t = sbw.tile([96, 2, CP], FP8, tag="wq", name="wq")
            wkv_t = sbw.tile([96, 2, 2 * CP], FP8, tag="wkv", name="wkv")
            nc.sync.dma_start(wq_t[:], d_wq.ap())
            qdw_t = [sbw.tile([128, 9, 128], FP8, tag=f"qdw{m}", name=f"qdw{m}") for m in range(2)]
            kvdw_t = [sbw.tile([128, 9, 128], FP8, tag=f"kvdw{m}", name=f"kvdw{m}") for m in range(4)]
            id128 = sbw.tile([128, 128], F32, tag="id128", name="id128")
            for m in range(2):
                nc.sync.dma_start(qdw_t[m][:], d_qdw[128 * m:128 * (m + 1)])
            nc.sync.dma_start(id128[:], d_id128.ap())

            def load_kv_weights():
                nc.sync.dma_start(wkv_t[:], d_wkv.ap())
                for m in range(4):
                    nc.sync.dma_start(kvdw_t[m][:], d_kvdw[128 * m:128 * (m + 1)])
            wlin_t = sbw.tile([128, 2, C], BF16, tag="wlin", name="wlin")
            wf1_t = sbw.tile([96, 2, 768], FP8, tag="wf1", name="wf1")
            wf2_t = sbw.tile([128, 3, 2, C], FP8, tag="wf2", name="wf2")
            wpr_t = sbw.tile([96, 2, C], BF16, tag="wpr", name="wpr")
            tempb = sbw.tile([128, 2], F32, tag="tempb", name="tempb")
            gscv = sbw.tile([128, 1], F32, tag="gscv", name="gscv")

            def load_trunk_weights():
                nc.sync.dma_start(wlin_t[:], d_wlin.ap())
                nc.sync.dma_start(wf1_t[:], d_wf1.ap())
                nc.sync.dma_start(wf2_t[:], d_wf2.ap())
                nc.sync.dma_start(wpr_t[:], d_wpr.ap())
                nc.sync.dma_start(tempb[:], d_tempb.ap())
                nc.sync.dma_start(gscv[:], d_gamma.ap())

            # persistent attn-path results; qkT packs q (cols 0:112) and
            # k (cols 112:224) transposed per band-pair
            qkT = [sbT.tile([128, 64, 224], BF16, tag=f"qkT{p}", name=f"qkT{p}")
                   for p in range(2)]
            vband = [sbT.tile([128, NLOC], BF16, tag=f"v{m}", name=f"v{m}") for m in range(2)]
            # gram + norm accumulators live in the tp-tag PSUM (idle pre-trunk)
            qgacc = pcv.tile([112, 448], F32, tag="tp", bufs=2, name="qgacc")
            kgacc = pcv.tile([112, 224], F32, tag="tp", bufs=2, name="kgacc")

            # ============ q/k/v production ============
            def band_producer(src_dram, w_t, dw_tiles, m, sink,
                              collect_mms=None):
                """One 128-wide band: conv1x1 (fp8 DR) + depthwise 3x3.
                Returns (need_groups, emit_dw) for interleaved emission."""
                pre = sbpre.tile([128, ER, EC], FP8, tag="pre", name="pre")
                pref = pre[:].rearrange("p a b -> p (a b)")
                state = {"g": 0, "ci": 0}

                def need_groups(ng):
                    while state["g"] < min(ng, len(CONV_GROUPS)):
                        g0, gn = CONV_GROUPS[state["g"]]
                        xc = sbin.tile([96, GRP, 2], FP8, tag="xin", name="xin")
                        nc.sync.dma_start(xc[:, :gn, :],
                                          src_dram[:, g0:g0 + gn, :])
                        for c0 in range(0, gn, 512):
                            cn = min(512, gn - c0)
                            ps = pcv.tile([128, 512], F32, tag="cv", name="cv")
                            mm = nc.tensor.matmul(
                                ps[:, :cn],
                                w_t[:, :, 128 * m:128 * (m + 1)],
                                xc[:, c0:c0 + cn, :].rearrange(
                                    "p n two -> p two n"),
                                start=True, stop=True, perf_mode=DRM)
                            if collect_mms is not None:
                                collect_mms.append(mm)
                            if state["ci"] % 2 == 0:
                                nc.vector.tensor_copy(
                                    pref[:, g0 + c0:g0 + c0 + cn], ps[:, :cn])
                            else:
                                nc.scalar.copy(
                                    pref[:, g0 + c0:g0 + c0 + cn], ps[:, :cn])
                            state["ci"] += 1
                        state["g"] += 1

                def emit_dw(ck):
                    r0 = 1 + 4 * ck
                    dp = pdw.tile([128, 4, 128], F32, tag="dw", name="dw")
                    for i, (ta, tb, stride) in enumerate(DW_PAIRS):
                        dra, dca = ta // 3 - 1, ta % 3 - 1
                        base = pre[:, r0 + dra:r0 + 4 + dra,
                                   1 + dca:129 + dca]
                        nc.tensor.matmul(
                            dp[:], dw_tiles[m][:, 2 * i:2 * i + 2, :],
                            _pair_ap(base, stride),
                            start=(i == 0), stop=False, perf_mode=DRM)
                    nc.tensor.matmul(
                        dp[:], dw_tiles[m][:, 8, :],
                        pre[:, r0:r0 + 4, 1:129],
                        start=False, stop=True)
                    sink(m, ck, dp[:].rearrange("p a b -> p (a b)"))

                return need_groups, emit_dw

            def conv_dw_path(src_dram, w_t, dw_tiles, n_mb, sink, m_off=0,
                             collect_mms=None):
                for m in range(m_off, m_off + n_mb):
                    ng, edw = band_producer(src_dram, w_t, dw_tiles, m, sink,
                                            collect_mms)
                    ng(len(CONV_GROUPS))
                    for ck in range(NCK if KSUB >= 2 else 0):
                        edw(ck)

            def qk_sink(coff):
                qcbig = [None]

                def sink(m, ck, flat):
                    j = ck % 4
                    if j == 0:
                        qcbig[0] = sbqk.tile([128, 2048], BF16, tag="qkc", name="qkc")
                    qc = qcbig[0][:, 512 * j:512 * (j + 1)]
                    if ck % 2 == 0:
                        nc.vector.tensor_copy(qc, flat)
                    else:
                        nc.scalar.copy(qc, flat)
                    if KSUB >= 4 and j == 3:
                        nc.sync.dma_start_transpose(
                            qkT[m][:, 4 * ck - 12:4 * ck + 4, coff:coff + 112],
                            qcbig[0][0:112, :])
                return sink

            def v_sink(m, ck, flat):
                dst = vband[m - 2]
                if ck % 2 == 0:
                    nc.vector.tensor_copy(dst[:, ck * 512:(ck + 1) * 512], flat)
                else:
                    nc.scalar.copy(dst[:, ck * 512:(ck + 1) * 512], flat)

            sinkq = qk_sink(0)
            sinkk = qk_sink(112)
            sqv = sbs.tile([128, 2], F32, tag="sqv", name="sqv")
            skv = sbs.tile([128, 2], F32, tag="skv", name="skv")

            def run_band(src, w_t, dwt, m, sink):
                ng, edw = band_producer(src, w_t, dwt, m, sink)
                ng(len(CONV_GROUPS))
                for ck in range(NCK if KSUB >= 2 else 0):
                    edw(ck)

            def gram_chunks(p, ck0, ck1):
                # gram + q/k sq-norms for band-pair p (qg: [q^T q | q^T k],
                # kg: k^T k)
                for ck in range(ck0, ck1):
                    nc.tensor.matmul(qgacc[:, 224 * p:224 * (p + 1)],
                                     qkT[p][:, ck, 0:112], qkT[p][:, ck, :],
                                     start=(ck == 0), stop=(ck == 63))
                    nc.tensor.matmul(kgacc[:, 112 * p:112 * (p + 1)],
                                     qkT[p][:, ck, 112:224],
                                     qkT[p][:, ck, 112:224],
                                     start=(ck == 0), stop=(ck == 63))

            def finish_ar(p):
                dsc = sbs.tile([112, 112], F32, tag="dsc", name="dsc")
                nc.vector.tensor_tensor(
                    out=dsc[:], in0=qgacc[:, 224 * p:224 * p + 112],
                    in1=id128[0:112, 0:112], op=OP.mult)
                nc.vector.tensor_reduce(sqv[0:112, p:p + 1], dsc[:],
                                        axis=mybir.AxisListType.X, op=OP.add)
                dsc2 = sbs.tile([112, 112], F32, tag="dsc2", name="dsc2")
                nc.vector.tensor_tensor(
                    out=dsc2[:], in0=kgacc[:, 112 * p:112 * (p + 1)],
                    in1=id128[0:112, 0:112], op=OP.mult)
                nc.vector.tensor_reduce(skv[0:112, p:p + 1], dsc2[:],
                                        axis=mybir.AxisListType.X, op=OP.add)
                gsb = sbs.tile([112, 114], F32, tag=f"gsb{p}", name=f"gsb{p}")
                nc.vector.tensor_copy(gsb[:, 0:112],
                                      qgacc[:, 224 * p + 112:224 * (p + 1)])
                nc.scalar.dma_start(cc_in[p].ap()[:, 0:112], gsb[:, 0:112])
                nc.scalar.dma_start(cc_in[p].ap()[:, 112:113], sqv[0:112, p:p + 1])
                nc.scalar.dma_start(cc_in[p].ap()[:, 113:114], skv[0:112, p:p + 1])
                nc.gpsimd.collective_compute(
                    "AllReduce", OP.add,
                    replica_groups=[[0, 1], [2, 3], [4, 5], [6, 7]],
                    ins=[cc_in[p].ap()], outs=[cc_out[p].ap()])

            run_band(d_xe, wq_t, qdw_t, 0, sinkq)
            load_kv_weights()
            run_band(d_ye, wkv_t, kvdw_t, 0, sinkk)
            # q band 1 with gram p0 interleaved (needs only q0/k0)
            ngq1, edwq1 = band_producer(d_xe, wq_t, qdw_t, 1, sinkq)
            ngq1(len(CONV_GROUPS))
            for ck in range(NCK if KSUB >= 2 else 0):
                edwq1(ck)
                if STAGE >= 3:
                    gram_chunks(0, 4 * ck, 4 * ck + 4)
            if STAGE >= 3:
                finish_ar(0)
            run_band(d_ye, wkv_t, kvdw_t, 1, sinkk)
            load_trunk_weights()

            if STAGE < 3:
                oc0 = sbs.tile([128, 2], F32, tag="oc0d", name="oc0d")
                nc.vector.tensor_copy(oc0[:], sqv[:])
                nc.sync.dma_start(d_out[0:128, 0:2], oc0[:])
            if STAGE >= 3:
                # v(m2) with gram p1 interleaved, + v(m3) head start —
                # these overlap the AllReduce wire time
                v_mms = []
                ngv2, edwv2 = band_producer(d_ye, wkv_t, kvdw_t, 2, v_sink,
                                            collect_mms=v_mms)
                ngv2(len(CONV_GROUPS))
                for ck in range(NCK):
                    edwv2(ck)
                    gram_chunks(1, 4 * ck, 4 * ck + 4)
                finish_ar(1)
                vng3, vdw3 = band_producer(d_ye, wkv_t, kvdw_t, 3, v_sink,
                                           collect_mms=v_mms)
                vng3(len(CONV_GROUPS))
                for ck in range(6):
                    vdw3(ck)

                gg = sbs.tile([112, 224], F32, tag="gg", name="gg")
                sqg = sbs.tile([128, 2], F32, tag="sqg", name="sqg")
                skg = sbs.tile([128, 2], F32, tag="skg", name="skg")
                nc.vector.memset(sqg[:], 1.0)
                nc.vector.memset(skg[:], 1.0)
                for p in range(2):
                    nc.sync.dma_start(gg[:, 112 * p:112 * (p + 1)],
                                      cc_out[p].ap()[:, 0:112])
                    nc.sync.dma_start(sqg[0:112, p:p + 1],
                                      cc_out[p].ap()[:, 112:113])
                    nc.sync.dma_start(skg[0:112, p:p + 1],
                                      cc_out[p].ap()[:, 113:114])

                # ============ attention finalize ============
                def rsqrt_newton(tag, s_t):
                    sc = sbs.tile([128, 2], F32, tag=tag + "_c")
                    nc.vector.tensor_scalar_max(sc[:], s_t[:], 1e-24)
                    rt = sbs.tile([128, 2], F32, tag=tag + "_s")
                    nc.scalar.activation(rt[:], sc[:], AF.Sqrt)
                    r0 = sbs.tile([128, 2], F32, tag=tag + "_r0")
                    nc.vector.reciprocal(r0[:], rt[:])
                    rr = sbs.tile([128, 2], F32, tag=tag + "_rr")
                    nc.vector.tensor_tensor(out=rr[:], in0=r0[:], in1=r0[:], op=OP.mult)
                    t1_ = sbs.tile([128, 2], F32, tag=tag + "_t1")
                    nc.vector.scalar_tensor_tensor(out=t1_[:], in0=sc[:], scalar=-0.5,
                                                   in1=rr[:], op0=OP.mult, op1=OP.mult)
                    nc.vector.tensor_scalar_add(t1_[:], t1_[:], 1.5)
                    rv = sbs.tile([128, 2], F32, tag=tag)
                    nc.vector.tensor_tensor(out=rv[:], in0=r0[:], in1=t1_[:], op=OP.mult)
                    return rv

                rq = rsqrt_newton("rq", sqg)
                rk = rsqrt_newton("rk", skg)
                srow = sbs.tile([128, 2], F32, tag="srow", name="srow")
                nc.vector.tensor_tensor(out=srow[:], in0=rq[:], in1=tempb[:], op=OP.mult)

                srow_r, scol_r = [], []
                for p in range(2):
                    for src, lst, nm in ((srow, srow_r, "sr"), (rk, scol_r, "sc")):
                        fp = psm.tile([1, 112], F32, tag="sm", name="sm")
                        nc.tensor.transpose(fp[:], src[0:112, p:p + 1],
                                            id128[0:112, 0:112])
                        fr = sbs.tile([1, 112], F32R, tag=f"{nm}{p}", name=f"{nm}{p}")
                        nc.vector.tensor_copy(fr[:], fp[:])
                        lst.append(fr)

                # W_comb = beta*lin*attn, [v-ch(pad 128), plane p, out 192];
                # pad v rows stay zero (vband pad rows are zero anyway)
                wcs = sbs.tile([128, 2, C], BF16, tag="wcs", name="wcs")
                nc.gpsimd.memset(wcs[:], 0.0)
                for p in range(2):
                    spair = psm.tile([112, 112], F32, tag="sm", name="sm")
                    nc.tensor.matmul(spair[:], srow_r[p][:], scol_r[p][:],
                                     start=True, stop=True)
                    lg = sbs.tile([112, 112], F32, tag="lg", name="lg")
                    nc.vector.tensor_tensor(out=lg[:], in0=gg[:, 112 * p:112 * (p + 1)],
                                            in1=spair[:], op=OP.mult)
                    at16 = sbs.tile([112, 112], BF16, tag="at16", name="at16")
                    for e in range(2):
                        sl = slice(64 * e, 64 * e + 48)
                        mx = sbs.tile([112, 1], F32, tag="mx", name="mx")
                        nc.vector.tensor_reduce(mx[sl, :], lg[sl, sl],
                                                axis=mybir.AxisListType.X, op=OP.max)
                        exh = sbs.tile([112, 112], F32, tag="exh", name="exh")
                        nc.vector.tensor_scalar(out=exh[sl, 0:48], in0=lg[sl, sl],
                                                scalar1=mx[sl, :], scalar2=None,
                                                op0=OP.subtract)
                        ex2 = sbs.tile([112, 112], F32, tag="ex2", name="ex2")
                        den = sbs.tile([112, 1], F32, tag="den", name="den")
                        nc.scalar.activation(ex2[sl, 0:48], exh[sl, 0:48], AF.Exp,
                                             accum_out=den[sl, :])
                        rc0 = sbs.tile([112, 1], F32, tag="rc0", name="rc0")
                        nc.vector.reciprocal(rc0[sl, :], den[sl, :])
                        nt = sbs.tile([112, 1], F32, tag="nt", name="nt")
                        nc.vector.tensor_tensor(out=nt[sl, :], in0=den[sl, :],
                                                in1=rc0[sl, :], op=OP.mult)
                        nc.vector.tensor_scalar(out=nt[sl, :], in0=nt[sl, :],
                                                scalar1=-1.0, scalar2=2.0,
                                                op0=OP.mult, op1=OP.add)
                        rc1 = sbs.tile([112, 1], F32, tag="rc1", name="rc1")
                        nc.vector.tensor_tensor(out=rc1[sl, :], in0=rc0[sl, :],
                                                in1=nt[sl, :], op=OP.mult)
                        nc.vector.tensor_scalar(out=at16[sl, 0:48], in0=ex2[sl, 0:48],
                                                scalar1=rc1[sl, :], scalar2=None,
                                                op0=OP.mult)
                        wcp = pcv.tile([128, 512], F32, tag="cv", name="cv")
                        nc.tensor.matmul(wcp[sl, :C], at16[sl, 0:48],
                                         wlin_t[64 * e:64 * e + 48, p, :],
                                         start=True, stop=True)
                        nc.vector.tensor_copy(wcs[sl, p, :], wcp[sl, :C])

                if STAGE < 4:
                    for ck in range(6, NCK):
                        vdw3(ck)
                    ocx = sbs.tile([112, 64], F32, tag="ocx", name="ocx")
                    nc.vector.tensor_copy(ocx[:], wcs[0:112, 0, 0:64])
                    nc.sync.dma_start(d_out[0:112, 0:64], ocx[:])
                # ==== software-pipelined trunk, interleaved with v band-3 ====
                # stage A(ck): v-dw chunk (6 ahead) + tp matmuls — one iter ahead
                # stage B(ck): t1 -> ffn1 -> gelu -> ffn2 -> t2 -> proj
                tp_q, t1_q = {}, {}

                def stage_a(ck):
                    if ck + 6 < NCK:
                        vdw3(ck + 6)
                    c0 = ck * 512
                    # fused z+lin: tp = W_comb^T v = 256x t'_true
                    tp = [pcv.tile([128, 512], F32, tag="tp", bufs=2,
                                   name="tp") for _ in range(2)]
                    for mi in range(2):
                        for p in range(2):
                            nc.tensor.matmul(tp[mi][:96, :],
                                             wcs[:, p, 96 * mi:96 * (mi + 1)],
                                             vband[p][:, c0:c0 + 512],
                                             start=(p == 0), stop=(p == 1))
                    tp_q[ck] = tp

                def stage_t1(ck):
                    tp = tp_q.pop(ck)
                    c0 = ck * 512
                    ycn = sbc.tile([96, 2, 512], BF16, tag="ycn", name="ycn")
                    for mi in range(2):
                        nc.sync.dma_start(ycn[:, mi, :],
                                          d_yc[96 * mi:96 * (mi + 1), c0:c0 + 512])
                    # t1f = gamma*t1 = ycn(= g*a*y) + tp * (gamma/256)
                    t1f = sbc.tile([96, 2, 512], BF16, tag="t1f", name="t1f")
                    for mi in range(2):
                        nc.vector.scalar_tensor_tensor(
                            out=t1f[:, mi, :], in0=tp[mi][:96, :],
                            scalar=gscv[0:96, :],
                            in1=ycn[:, mi, :], op0=OP.mult, op1=OP.add)
                    t1c8 = sbc.tile([96, 512, 2], FP8, tag="t1c8", name="t1c8")
                    nc.vector.tensor_copy(
                        t1c8[:].rearrange("p n two -> p two n"), t1f[:])
                    t1_q[ck] = (t1f, t1c8)

                def stage_b(ck):
                    t1f, t1c8 = t1_q.pop(ck)
                    c0 = ck * 512
                    # ffn1 + gelu: fp1 = 8x f1_true; gc8 = gelu(f1_true)
                    gc8 = [sbg.tile([128, 512, 2], FP8, tag="gc8", name="gc8")
                           for _ in range(3)]
                    for mt in range(6):
                        fp1 = pcv.tile([128, 512], F32, tag="cv", name="cv")
                        nc.tensor.matmul(fp1[:],
                                         wf1_t[:, :, 128 * mt:128 * (mt + 1)],
                                         t1c8[:].rearrange("p n two -> p two n"),
                                         start=True, stop=True,
                                         perf_mode=DRM)
                        nc.scalar.activation(gc8[mt // 2][:, :, mt % 2], fp1[:],
                                             AF.Gelu, scale=0.125)
                    # ffn2: fp2 = 8*delta*f2_true; t2 = t1f + fp2/8
                    t2c = sbc.tile([96, 2, 512], BF16, tag="t2c", name="t2c")
                    for mi in range(2):
                        fp2 = pcv.tile([128, 512], F32, tag="cv", name="cv")
                        for jp in range(3):
                            nc.tensor.matmul(fp2[:96, :],
                                             wf2_t[:, jp, :, 96 * mi:96 * (mi + 1)],
                                             gc8[jp][:].rearrange(
                                                 "p n two -> p two n"),
                                             start=(jp == 0),
                                             stop=(jp == 2), perf_mode=DRM)
                        nc.vector.scalar_tensor_tensor(
                            out=t2c[:, mi, :], in0=fp2[:96, :], scalar=0.125,
                            in1=t1f[:, mi, :], op0=OP.mult, op1=OP.add)
                    # proj (bf16): accumulate over the two 96-ch planes
                    for mi, (mo, ms) in enumerate(KB):
                        pp = pcv.tile([128, 512], F32, tag="cv", name="cv")
                        for pl in range(2):
                            nc.tensor.matmul(pp[:ms, :],
                                             wpr_t[:, pl, mo:mo + ms],
                                             t2c[:, pl, :],
                                             start=(pl == 0), stop=(pl == 1))
                        oc = sbc.tile([128, 512], F32, tag=f"oc{mi}", name=f"oc{mi}")
                        if mi == 0:
                            nc.scalar.copy(oc[:ms, :], pp[:ms, :])
                        else:
                            nc.vector.tensor_copy(oc[:ms, :], pp[:ms, :])
                        nc.sync.dma_start(d_out[mo:mo + ms, c0:c0 + 512],
                                          oc[:ms, :])

                if STAGE >= 4:
                    stage_a(0)
                    stage_t1(0)
                    for ck in range(NCK):
                        if ck + 1 < NCK:
                            stage_a(ck + 1)
                        stage_b(ck)
                        if ck + 1 < NCK:
                            stage_t1(ck + 1)

    nc.compile()
    return nc


_NC = None


def _get_nc():
    global _NC
    if _NC is None:
        _NC = build_nc()
    return _NC


def _prep_weights(q_w, q_dw_w, kv_w, kv_dw_w, linear_w, proj_w, ffn1_w, ffn2_w,
                  temperature, alpha, beta, gamma, delta):
    def pad_oc(w):  # [192 real oc, ic] -> [ic, 256 padded oc]
        out = np.zeros((C, CP), np.float32)
        for h in range(HEADS):
            out[:, CPH * h:CPH * h + CH] = w[CH * h:CH * (h + 1), :].T
        return out

    wq = pad_oc(np.asarray(q_w, np.float32)) * 8.0
    kv = np.asarray(kv_w, np.float32)
    wkv = np.concatenate([pad_oc(kv[:C]), pad_oc(kv[C:])], axis=1) * 8.0

    # [192,1,3,3] -> [256, 9, 128] diag, slots = DW_PAIRS order + center
    slot_tap = [0, 2, 3, 5, 6, 8, 1, 7, 4]

    def pad_dw(w):
        out = np.zeros((CP, 9, 128), np.float32)
        for h in range(HEADS):
            for j in range(CH):
                cp = CPH * h + j
                taps = w[CH * h + j, 0].reshape(9)
                for s, t in enumerate(slot_tap):
                    out[cp, s, cp % 128] = taps[t]
        return out * 32.0

    qdw = pad_dw(np.asarray(q_dw_w, np.float32))
    kvd = np.asarray(kv_dw_w, np.float32)
    kvdw = np.concatenate([pad_dw(kvd[:C]), pad_dw(kvd[C:])], axis=0)

    gamma_f = float(gamma)
    # wlin (bf16) = beta*lin padded; W_comb = attn x wlin -> tp = 256x t'_true
    lin = np.asarray(linear_w, np.float32) * float(beta)
    wlin = np.zeros((CP, C), np.float32)
    for h in range(HEADS):
        wlin[CPH * h:CPH * h + CH, :] = lin[:, CH * h:CH * (h + 1)].T
    wlin8 = wlin.reshape(2, 128, C).transpose(1, 0, 2)

    # t1c8 = gamma*t1; wf18 = ffn1_w^T * 8/gamma -> fp1 = 8x f1_true
    wf1 = np.asarray(ffn1_w, np.float32).T * (8.0 / gamma_f)
    wf18 = wf1.reshape(2, 96, 768).transpose(1, 0, 2)
    # wf28 = ffn2_w^T * 8*delta -> fp2 = 8*delta*f2_true
    wf2 = np.asarray(ffn2_w, np.float32).T * (8.0 * float(delta))
    wf28 = wf2.reshape(3, 2, 128, C).transpose(2, 0, 1, 3)
    wpr = np.asarray(proj_w, np.float32).T.reshape(2, 96, C).transpose(1, 0, 2)

    tempb = np.zeros((128, 2), np.float32)
    tv = np.asarray(temperature, np.float32).reshape(HEADS)
    for h in range(HEADS):
        tempb[64 * (h % 2):64 * (h % 2) + 64, h // 2] = tv[h]

    gscv = np.full((128, 1), gamma_f / 256.0, np.float32)
    id128 = np.eye(128, dtype=np.float32)

    return {
        "_yscale": gamma_f * float(alpha),
        "wq": wq.reshape(2, 96, CP).transpose(1, 0, 2).astype(f8),
        "wkv": wkv.reshape(2, 96, 2 * CP).transpose(1, 0, 2).astype(f8),
        "qdw": qdw.astype(f8), "kvdw": kvdw.astype(f8),
        "wlin": wlin8.astype(bf16).copy(), "wf1": wf18.astype(f8),
        "wf2": wf28.astype(f8), "wpr": wpr.astype(bf16).copy(),
        "tempb": tempb, "gamma": gscv,
        "id128": id128,
    }


def _make_in_maps(x, y, shared):
    shared = dict(shared)
    yscale = shared.pop("_yscale")
    in_maps = []
    for c in range(N_CORES):
        bi, s = c // 2, c % 2
        r0 = s * HLOC
        xe = np.zeros((C, ER, EC), np.float32)
        ye = np.zeros((C, ER, EC), np.float32)
        rlo, rhi = max(r0 - 1, 0), min(r0 + HLOC + 1, H)
        elo = rlo - (r0 - 1)
        xe[:, elo:elo + (rhi - rlo), 1:129] = x[bi, :, rlo:rhi, :]
        ye[:, elo:elo + (rhi - rlo), 1:129] = y[bi, :, rlo:rhi, :]
        m = dict(shared)
        m["xe"] = xe.reshape(2, 96, NEXT).transpose(1, 2, 0).astype(f8)
        m["ye"] = ye.reshape(2, 96, NEXT).transpose(1, 2, 0).astype(f8)
        m["yc"] = (yscale * y[bi, :, r0:r0 + HLOC, :]
                   ).reshape(C, NLOC).astype(bf16)
        in_maps.append(m)
    return in_maps


def kernel(**inputs):
    x = np.asarray(inputs["x"], np.float32)
    y = np.asarray(inputs["y"], np.float32)
    shared = _prep_weights(
        inputs["q_w"], inputs["q_dw_w"], inputs["kv_w"], inputs["kv_dw_w"],
        inputs["linear_w"], inputs["proj_w"], inputs["ffn1_w"], inputs["ffn2_w"],
        inputs["temperature"], inputs["alpha"], inputs["beta"],
        inputs["gamma"], inputs["delta"])

    in_maps = _make_in_maps(x, y, shared)

    nc = _get_nc()
    res = run_bass_kernel_spmd(nc, in_maps, list(range(N_CORES)))
    out = np.empty((B, C, H, W), np.float32)
    for c in range(N_CORES):
        bi, s = c // 2, c % 2
        out[bi, :, s * HLOC:(s + 1) * HLOC, :] = \
            res.results[c]["out"].reshape(C, HLOC, W)
    return out



# revision 73
# speedup vs baseline: 1.1402x; 1.0148x over previous
"""Trainium2 Bass kernel for nn_CDEM_62079457296798 (channel-attention
transformer block).

Sharding: 8 cores = 4 batches x 2 spatial halves (64 rows + 1 halo row each).
Cross-core communication: two small per-band-pair AllReduces carrying the
channel-attention Gram matrices and q/k l2-norm sums; everything else local.

Layout: channel-major activations [C_part, pixels_free]; attention channels
padded 48 -> 64 per head. Heavy use of fp8e4m3 DoubleRow matmuls (2 K-planes
per instruction; planes interleaved in memory so the PE streams 2 rows/cycle):
the q/kv 1x1 convs pair the 192 input channels as [96, 2]; the depthwise 3x3
runs as 4 tap-pair DoubleRow matmuls with diagonal [128, 2, 128] weights
(overlapping-stride pair APs over the padded image) + 1 bf16-free center tap;
ffn1/ffn2 pair K the same way. q/k sq-norms ride the Gram matmuls
(qg = q^T [q|k], kg = k^T k; diag extracted via masked reduce). z and lin are
fused: W_comb = beta*lin*attn is built once after softmax, so the per-chunk
trunk is W_comb^T v -> t1 -> ffn -> proj (proj in bf16). All runtime scalars
(alpha/beta/gamma/delta) are folded host-side; kernel-side rescales are
compile-time powers of two. The trunk is software-pipelined (tp one chunk
ahead) and interleaved with v-band production to keep the PE stream dense.
"""
import sys
sys.path.insert(0, '/opt/trn_rl_repo')

import numpy as np
import ml_dtypes

import bass_rust
from concourse import bacc, mybir, tile
from concourse.bass import _add_dep_helper
from concourse.bass_utils import run_bass_kernel_spmd

F32 = mybir.dt.float32
F32R = mybir.dt.float32r
BF16 = mybir.dt.bfloat16
FP8 = mybir.dt.float8e4
DRM = mybir.MatmulPerfMode.DoubleRow
AF = mybir.ActivationFunctionType
OP = mybir.AluOpType
bf16 = ml_dtypes.bfloat16
f8 = ml_dtypes.float8_e4m3fn

# depthwise 3x3 as 4 fp8 DoubleRow pairs + 1 single (tap index t = 3*(dr+1)+(dc+1));
# pair strides in elements of the [ER, EC] image (2 = two cols, 260 = two rows)
DW_PAIRS = [(0, 2, 2), (3, 5, 2), (6, 8, 2), (1, 7, 2 * 130)]
DW_SINGLE = 4


def _pair_ap(base, stride):
    raw = base.ap.copy()
    return bass_rust.AP(base.tensor, base.offset,
                        [raw[0], [stride, 2]] + list(raw[1:]))

N_CORES = 8
B, C, H, W = 4, 192, 128, 128
HEADS, CH = 4, 48
CPH = 64                # padded channels per head
CP = HEADS * CPH        # 256 padded attn channels
HLOC = 64               # image rows per core
ER, EC = 66, 130        # ext rows/cols (halo + zero pad)
NEXT = ER * EC          # 8580
NLOC = HLOC * W         # 8192
NCK = 16                # output chunks (4 rows x 128 = 512 px)
CONV_CHUNKS = [(i * 512, 512) for i in range(16)] + [(16 * 512, NEXT - 16 * 512)]
GRP = 2048
CONV_GROUPS = [(i * GRP, GRP) for i in range(4)] + [(4 * GRP, NEXT - 4 * GRP)]
KB = [(0, 128), (128, 64)]          # 192-channel K bands

DIRECT_PSUM_OUT = False  # DMA cannot read PSUM on TRN2


import os
STAGE = int(os.environ.get("KSTAGE", "4"))
KSUB = int(os.environ.get("KSUB", "4"))


class _StageDone(Exception):
    pass


def build_nc():
    nc = bacc.Bacc("TRN2", target_bir_lowering=False, debug=False,
                   num_devices=N_CORES)

    d_xe = nc.dram_tensor("xe", [96, NEXT, 2], FP8, kind="ExternalInput")
    d_ye = nc.dram_tensor("ye", [96, NEXT, 2], FP8, kind="ExternalInput")
    d_yc = nc.dram_tensor("yc", [C, NLOC], BF16, kind="ExternalInput")
    d_wq = nc.dram_tensor("wq", [96, 2, CP], FP8, kind="ExternalInput")
    d_wkv = nc.dram_tensor("wkv", [96, 2, 2 * CP], FP8, kind="ExternalInput")
    d_qdw = nc.dram_tensor("qdw", [CP, 9, 128], FP8, kind="ExternalInput")
    d_kvdw = nc.dram_tensor("kvdw", [2 * CP, 9, 128], FP8, kind="ExternalInput")
    d_wlin = nc.dram_tensor("wlin", [128, 2, C], BF16, kind="ExternalInput")
    d_wf1 = nc.dram_tensor("wf1", [96, 2, 768], FP8, kind="ExternalInput")
    d_wf2 = nc.dram_tensor("wf2", [128, 3, 2, C], FP8, kind="ExternalInput")
    d_wpr = nc.dram_tensor("wpr", [96, 2, C], BF16, kind="ExternalInput")
    d_tempb = nc.dram_tensor("tempb", [128, 2], F32, kind="ExternalInput")
    d_gamma = nc.dram_tensor("gamma", [128, 1], F32, kind="ExternalInput")
    d_id128 = nc.dram_tensor("id128", [128, 128], F32, kind="ExternalInput")
    d_out = nc.dram_tensor("out", [C, NLOC], F32, kind="ExternalOutput")
    cc_in = [nc.dram_tensor(f"cc_in{p}", [112, 114], F32) for p in range(2)]
    cc_out = [nc.dram_tensor(f"cc_out{p}", [112, 114], F32) for p in range(2)]

    with tile.TileContext(nc) as tc:
        with (
            tc.tile_pool(name="sbw", bufs=1) as sbw,      # weights/consts
            tc.tile_pool(name="sbpre", bufs=2) as sbpre,  # conv1x1 out (ext img)
            tc.tile_pool(name="sbin", bufs=5) as sbin,    # streamed conv inputs
            tc.tile_pool(name="sbqk", bufs=4) as sbqk,    # q/k chunk tiles
            tc.tile_pool(name="sbT", bufs=1) as sbT,      # qT/kT/v persistents
            tc.tile_pool(name="sbs", bufs=1) as sbs,      # small attn tiles
            tc.tile_pool(name="sbc", bufs=3) as sbc,      # trunk chunk pipeline
            tc.tile_pool(name="sbg", bufs=6) as sbg,      # gelu chunk tiles
            tc.tile_pool(name="pcv", bufs=3, space="PSUM") as pcv,
            tc.tile_pool(name="pdw", bufs=2, space="PSUM") as pdw,
            tc.tile_pool(name="psm", bufs=1, space="PSUM") as psm,
        ):
            # ---------- weights ----------
            wq_t = sbw.tile([96, 2, CP], FP8, tag="wq", name="wq")
            wkv_t = sbw.tile([96, 2, 2 * CP], FP8, tag="wkv", name="wkv")
            nc.sync.dma_start(wq_t[:], d_wq.ap())
            qdw_t = [sbw.tile([128, 9, 128], FP8, tag=f"qdw{m}", name=f"qdw{m}") for m in range(2)]
            kvdw_t = [sbw.tile([128, 9, 128], FP8, tag=f"kvdw{m}", name=f"kvdw{m}") for m in range(4)]
            id128 = sbw.tile([128, 128], F32, tag="id128", name="id128")
            for m in range(2):
                nc.sync.dma_start(qdw_t[m][:], d_qdw[128 * m:128 * (m + 1)])
            nc.sync.dma_start(id128[:], d_id128.ap())

            def load_kv_weights():
                nc.sync.dma_start(wkv_t[:], d_wkv.ap())
                for m in range(4):
                    nc.sync.dma_start(kvdw_t[m][:], d_kvdw[128 * m:128 * (m + 1)])
            wlin_t = sbw.tile([128, 2, C], BF16, tag="wlin", name="wlin")
            wf1_t = sbw.tile([96, 2, 768], FP8, tag="wf1", name="wf1")
            wf2_t = sbw.tile([128, 3, 2, C], FP8, tag="wf2", name="wf2")
            wpr_t = sbw.tile([96, 2, C], BF16, tag="wpr", name="wpr")
            tempb = sbw.tile([128, 2], F32, tag="tempb", name="tempb")
            gscv = sbw.tile([128, 1], F32, tag="gscv", name="gscv")

            def load_trunk_weights():
                nc.sync.dma_start(wlin_t[:], d_wlin.ap())
                nc.sync.dma_start(wf1_t[:], d_wf1.ap())
                nc.sync.dma_start(wf2_t[:], d_wf2.ap())
                nc.sync.dma_start(wpr_t[:], d_wpr.ap())
                nc.sync.dma_start(tempb[:], d_tempb.ap())
                nc.sync.dma_start(gscv[:], d_gamma.ap())

            # persistent attn-path results; qkT packs q (cols 0:112) and
            # k (cols 112:224) transposed per band-pair
            qkT = [sbT.tile([128, 64, 224], BF16, tag=f"qkT{p}", name=f"qkT{p}")
                   for p in range(2)]
            vband = [sbT.tile([128, NLOC], BF16, tag=f"v{m}", name=f"v{m}") for m in range(2)]
            # gram + norm accumulators live in the tp-tag PSUM (idle pre-trunk)
            qgacc = pcv.tile([112, 448], F32, tag="tp", bufs=2, name="qgacc")
            kgacc = pcv.tile([112, 224], F32, tag="tp", bufs=2, name="kgacc")

            # ============ q/k/v production ============
            def band_producer(src_dram, w_t, dw_tiles, m, sink,
                              collect_mms=None):
                """One 128-wide band: conv1x1 (fp8 DR) + depthwise 3x3.
                Returns (need_groups, emit_dw) for interleaved emission."""
                pre = sbpre.tile([128, ER, EC], FP8, tag="pre", name="pre")
                pref = pre[:].rearrange("p a b -> p (a b)")
                state = {"g": 0, "ci": 0}

                def need_groups(ng):
                    while state["g"] < min(ng, len(CONV_GROUPS)):
                        g0, gn = CONV_GROUPS[state["g"]]
                        xc = sbin.tile([96, GRP, 2], FP8, tag="xin", name="xin")
                        nc.sync.dma_start(xc[:, :gn, :],
                                          src_dram[:, g0:g0 + gn, :])
                        for c0 in range(0, gn, 512):
                            cn = min(512, gn - c0)
                            ps = pcv.tile([128, 512], F32, tag="cv", name="cv")
                            mm = nc.tensor.matmul(
                                ps[:, :cn],
                                w_t[:, :, 128 * m:128 * (m + 1)],
                                xc[:, c0:c0 + cn, :].rearrange(
                                    "p n two -> p two n"),
                                start=True, stop=True, perf_mode=DRM)
                            if collect_mms is not None:
                                collect_mms.append(mm)
                            if state["ci"] % 2 == 0:
                                nc.vector.tensor_copy(
                                    pref[:, g0 + c0:g0 + c0 + cn], ps[:, :cn])
                            else:
                                nc.scalar.copy(
                                    pref[:, g0 + c0:g0 + c0 + cn], ps[:, :cn])
                            state["ci"] += 1
                        state["g"] += 1

                def emit_dw(ck):
                    r0 = 1 + 4 * ck
                    dp = pdw.tile([128, 4, 128], F32, tag="dw", name="dw")
                    for i, (ta, tb, stride) in enumerate(DW_PAIRS):
                        dra, dca = ta // 3 - 1, ta % 3 - 1
                        base = pre[:, r0 + dra:r0 + 4 + dra,
                                   1 + dca:129 + dca]
                        nc.tensor.matmul(
                            dp[:], dw_tiles[m][:, 2 * i:2 * i + 2, :],
                            _pair_ap(base, stride),
                            start=(i == 0), stop=False, perf_mode=DRM)
                    nc.tensor.matmul(
                        dp[:], dw_tiles[m][:, 8, :],
                        pre[:, r0:r0 + 4, 1:129],
                        start=False, stop=True)
                    sink(m, ck, dp[:].rearrange("p a b -> p (a b)"))

                return need_groups, emit_dw

            def conv_dw_path(src_dram, w_t, dw_tiles, n_mb, sink, m_off=0,
                             collect_mms=None):
                for m in range(m_off, m_off + n_mb):
                    ng, edw = band_producer(src_dram, w_t, dw_tiles, m, sink,
                                            collect_mms)
                    ng(len(CONV_GROUPS))
                    for ck in range(NCK if KSUB >= 2 else 0):
                        edw(ck)

            def qk_sink(coff):
                qcbig = [None]

                def sink(m, ck, flat):
                    j = ck % 4
                    if j == 0:
                        qcbig[0] = sbqk.tile([128, 2048], BF16, tag="qkc", name="qkc")
                    qc = qcbig[0][:, 512 * j:512 * (j + 1)]
                    if ck % 2 == 0:
                        nc.vector.tensor_copy(qc, flat)
                    else:
                        nc.scalar.copy(qc, flat)
                    if KSUB >= 4 and j == 3:
                        nc.sync.dma_start_transpose(
                            qkT[m][:, 4 * ck - 12:4 * ck + 4, coff:coff + 112],
                            qcbig[0][0:112, :])
                return sink

            def v_sink(m, ck, flat):
                dst = vband[m - 2]
                if ck % 2 == 0:
                    nc.vector.tensor_copy(dst[:, ck * 512:(ck + 1) * 512], flat)
                else:
                    nc.scalar.copy(dst[:, ck * 512:(ck + 1) * 512], flat)

            sinkq = qk_sink(0)
            sinkk = qk_sink(112)
            sqv = sbs.tile([128, 2], F32, tag="sqv", name="sqv")
            skv = sbs.tile([128, 2], F32, tag="skv", name="skv")

            def run_band(src, w_t, dwt, m, sink):
                ng, edw = band_producer(src, w_t, dwt, m, sink)
                ng(len(CONV_GROUPS))
                for ck in range(NCK if KSUB >= 2 else 0):
                    edw(ck)

            def gram_chunks(p, ck0, ck1):
                # gram + q/k sq-norms for band-pair p (qg: [q^T q | q^T k],
                # kg: k^T k)
                for ck in range(ck0, ck1):
                    nc.tensor.matmul(qgacc[:, 224 * p:224 * (p + 1)],
                                     qkT[p][:, ck, 0:112], qkT[p][:, ck, :],
                                     start=(ck == 0), stop=(ck == 63))
                    nc.tensor.matmul(kgacc[:, 112 * p:112 * (p + 1)],
                                     qkT[p][:, ck, 112:224],
                                     qkT[p][:, ck, 112:224],
                                     start=(ck == 0), stop=(ck == 63))

            def finish_ar(p):
                dsc = sbs.tile([112, 112], F32, tag="dsc", name="dsc")
                nc.vector.tensor_tensor(
                    out=dsc[:], in0=qgacc[:, 224 * p:224 * p + 112],
                    in1=id128[0:112, 0:112], op=OP.mult)
                nc.vector.tensor_reduce(sqv[0:112, p:p + 1], dsc[:],
                                        axis=mybir.AxisListType.X, op=OP.add)
                dsc2 = sbs.tile([112, 112], F32, tag="dsc2", name="dsc2")
                nc.vector.tensor_tensor(
                    out=dsc2[:], in0=kgacc[:, 112 * p:112 * (p + 1)],
                    in1=id128[0:112, 0:112], op=OP.mult)
                nc.vector.tensor_reduce(skv[0:112, p:p + 1], dsc2[:],
                                        axis=mybir.AxisListType.X, op=OP.add)
                gsb = sbs.tile([112, 114], F32, tag=f"gsb{p}", name=f"gsb{p}")
                nc.vector.tensor_copy(gsb[:, 0:112],
                                      qgacc[:, 224 * p + 112:224 * (p + 1)])
                nc.scalar.dma_start(cc_in[p].ap()[:, 0:112], gsb[:, 0:112])
                nc.scalar.dma_start(cc_in[p].ap()[:, 112:113], sqv[0:112, p:p + 1])
                nc.scalar.dma_start(cc_in[p].ap()[:, 113:114], skv[0:112, p:p + 1])
                nc.gpsimd.collective_compute(
                    "AllReduce", OP.add,
                    replica_groups=[[0, 1], [2, 3], [4, 5], [6, 7]],
                    ins=[cc_in[p].ap()], outs=[cc_out[p].ap()])

            run_band(d_xe, wq_t, qdw_t, 0, sinkq)
            load_kv_weights()
            run_band(d_ye, wkv_t, kvdw_t, 0, sinkk)
            # q band 1 with gram p0 interleaved (needs only q0/k0)
            ngq1, edwq1 = band_producer(d_xe, wq_t, qdw_t, 1, sinkq)
            ngq1(len(CONV_GROUPS))
            for ck in range(NCK if KSUB >= 2 else 0):
                edwq1(ck)
                if STAGE >= 3:
                    gram_chunks(0, 4 * ck, 4 * ck + 4)
            # prefetch k1's first conv groups BEFORE the AR dispatch: the
            # collective wire phase blocks input DMAs for ~15us
            ngk1, edwk1 = band_producer(d_ye, wkv_t, kvdw_t, 1, sinkk)
            ngk1(3)
            if STAGE >= 3:
                finish_ar(0)
            ngk1(len(CONV_GROUPS))
            for ck in range(NCK if KSUB >= 2 else 0):
                edwk1(ck)
            load_trunk_weights()

            if STAGE < 3:
                oc0 = sbs.tile([128, 2], F32, tag="oc0d", name="oc0d")
                nc.vector.tensor_copy(oc0[:], sqv[:])
                nc.sync.dma_start(d_out[0:128, 0:2], oc0[:])
            if STAGE >= 3:
                # v(m2) with gram p1 interleaved, + v(m3) head start —
                # these overlap the AllReduce wire time
                v_mms = []
                ngv2, edwv2 = band_producer(d_ye, wkv_t, kvdw_t, 2, v_sink,
                                            collect_mms=v_mms)
                ngv2(len(CONV_GROUPS))
                for ck in range(NCK):
                    edwv2(ck)
                    gram_chunks(1, 4 * ck, 4 * ck + 4)
                vng3, vdw3 = band_producer(d_ye, wkv_t, kvdw_t, 3, v_sink,
                                           collect_mms=v_mms)
                vng3(3)
                finish_ar(1)
                vng3(len(CONV_GROUPS))
                for ck in range(6):
                    vdw3(ck)

                gg = sbs.tile([112, 224], F32, tag="gg", name="gg")
                sqg = sbs.tile([128, 2], F32, tag="sqg", name="sqg")
                skg = sbs.tile([128, 2], F32, tag="skg", name="skg")
                nc.vector.memset(sqg[:], 1.0)
                nc.vector.memset(skg[:], 1.0)
                for p in range(2):
                    nc.sync.dma_start(gg[:, 112 * p:112 * (p + 1)],
                                      cc_out[p].ap()[:, 0:112])
                    nc.sync.dma_start(sqg[0:112, p:p + 1],
                                      cc_out[p].ap()[:, 112:113])
                    nc.sync.dma_start(skg[0:112, p:p + 1],
                                      cc_out[p].ap()[:, 113:114])

                # ============ attention finalize ============
                def rsqrt_newton(tag, s_t):
                    sc = sbs.tile([128, 2], F32, tag=tag + "_c")
                    nc.vector.tensor_scalar_max(sc[:], s_t[:], 1e-24)
                    rt = sbs.tile([128, 2], F32, tag=tag + "_s")
                    nc.scalar.activation(rt[:], sc[:], AF.Sqrt)
                    r0 = sbs.tile([128, 2], F32, tag=tag + "_r0")
                    nc.vector.reciprocal(r0[:], rt[:])
                    rr = sbs.tile([128, 2], F32, tag=tag + "_rr")
                    nc.vector.tensor_tensor(out=rr[:], in0=r0[:], in1=r0[:], op=OP.mult)
                    t1_ = sbs.tile([128, 2], F32, tag=tag + "_t1")
                    nc.vector.scalar_tensor_tensor(out=t1_[:], in0=sc[:], scalar=-0.5,
                                                   in1=rr[:], op0=OP.mult, op1=OP.mult)
                    nc.vector.tensor_scalar_add(t1_[:], t1_[:], 1.5)
                    rv = sbs.tile([128, 2], F32, tag=tag)
                    nc.vector.tensor_tensor(out=rv[:], in0=r0[:], in1=t1_[:], op=OP.mult)
                    return rv

                rq = rsqrt_newton("rq", sqg)
                rk = rsqrt_newton("rk", skg)
                srow = sbs.tile([128, 2], F32, tag="srow", name="srow")
                nc.vector.tensor_tensor(out=srow[:], in0=rq[:], in1=tempb[:], op=OP.mult)

                srow_r, scol_r = [], []
                for p in range(2):
                    for src, lst, nm in ((srow, srow_r, "sr"), (rk, scol_r, "sc")):
                        fp = psm.tile([1, 112], F32, tag="sm", name="sm")
                        nc.tensor.transpose(fp[:], src[0:112, p:p + 1],
                                            id128[0:112, 0:112])
                        fr = sbs.tile([1, 112], F32R, tag=f"{nm}{p}", name=f"{nm}{p}")
                        nc.vector.tensor_copy(fr[:], fp[:])
                        lst.append(fr)

                # W_comb = beta*lin*attn, [v-ch(pad 128), plane p, out 192];
                # pad v rows stay zero (vband pad rows are zero anyway)
                wcs = sbs.tile([128, 2, C], BF16, tag="wcs", name="wcs")
                nc.gpsimd.memset(wcs[:], 0.0)
                for p in range(2):
                    spair = psm.tile([112, 112], F32, tag="sm", name="sm")
                    nc.tensor.matmul(spair[:], srow_r[p][:], scol_r[p][:],
                                     start=True, stop=True)
                    lg = sbs.tile([112, 112], F32, tag="lg", name="lg")
                    nc.vector.tensor_tensor(out=lg[:], in0=gg[:, 112 * p:112 * (p + 1)],
                                            in1=spair[:], op=OP.mult)
                    at16 = sbs.tile([112, 112], BF16, tag="at16", name="at16")
                    for e in range(2):
                        sl = slice(64 * e, 64 * e + 48)
                        mx = sbs.tile([112, 1], F32, tag="mx", name="mx")
                        nc.vector.tensor_reduce(mx[sl, :], lg[sl, sl],
                                                axis=mybir.AxisListType.X, op=OP.max)
                        exh = sbs.tile([112, 112], F32, tag="exh", name="exh")
                        nc.vector.tensor_scalar(out=exh[sl, 0:48], in0=lg[sl, sl],
                                                scalar1=mx[sl, :], scalar2=None,
                                                op0=OP.subtract)
                        ex2 = sbs.tile([112, 112], F32, tag="ex2", name="ex2")
                        den = sbs.tile([112, 1], F32, tag="den", name="den")
                        nc.scalar.activation(ex2[sl, 0:48], exh[sl, 0:48], AF.Exp,
                                             accum_out=den[sl, :])
                        rc0 = sbs.tile([112, 1], F32, tag="rc0", name="rc0")
                        nc.vector.reciprocal(rc0[sl, :], den[sl, :])
                        nt = sbs.tile([112, 1], F32, tag="nt", name="nt")
                        nc.vector.tensor_tensor(out=nt[sl, :], in0=den[sl, :],
                                                in1=rc0[sl, :], op=OP.mult)
                        nc.vector.tensor_scalar(out=nt[sl, :], in0=nt[sl, :],
                                                scalar1=-1.0, scalar2=2.0,
                                                op0=OP.mult, op1=OP.add)
                        rc1 = sbs.tile([112, 1], F32, tag="rc1", name="rc1")
                        nc.vector.tensor_tensor(out=rc1[sl, :], in0=rc0[sl, :],
                                                in1=nt[sl, :], op=OP.mult)
                        nc.vector.tensor_scalar(out=at16[sl, 0:48], in0=ex2[sl, 0:48],
                                                scalar1=rc1[sl, :], scalar2=None,
                                                op0=OP.mult)
                        wcp = pcv.tile([128, 512], F32, tag="cv", name="cv")
                        nc.tensor.matmul(wcp[sl, :C], at16[sl, 0:48],
                                         wlin_t[64 * e:64 * e + 48, p, :],
                                         start=True, stop=True)
                        nc.vector.tensor_copy(wcs[sl, p, :], wcp[sl, :C])

                if STAGE < 4:
                    for ck in range(6, NCK):
                        vdw3(ck)
                    ocx = sbs.tile([112, 64], F32, tag="ocx", name="ocx")
                    nc.vector.tensor_copy(ocx[:], wcs[0:112, 0, 0:64])
                    nc.sync.dma_start(d_out[0:112, 0:64], ocx[:])
                # ==== software-pipelined trunk, interleaved with v band-3 ====
                # stage A(ck): v-dw chunk (6 ahead) + tp matmuls — one iter ahead
                # stage B(ck): t1 -> ffn1 -> gelu -> ffn2 -> t2 -> proj
                tp_q, t1_q = {}, {}

                def stage_a(ck):
                    if ck + 6 < NCK:
                        vdw3(ck + 6)
                    c0 = ck * 512
                    # fused z+lin: tp = W_comb^T v = 256x t'_true
                    tp = [pcv.tile([128, 512], F32, tag="tp", bufs=2,
                                   name="tp") for _ in range(2)]
                    for mi in range(2):
                        for p in range(2):
                            nc.tensor.matmul(tp[mi][:96, :],
                                             wcs[:, p, 96 * mi:96 * (mi + 1)],
                                             vband[p][:, c0:c0 + 512],
                                             start=(p == 0), stop=(p == 1))
                    tp_q[ck] = tp

                def stage_t1(ck):
                    tp = tp_q.pop(ck)
                    c0 = ck * 512
                    ycn = sbc.tile([96, 2, 512], BF16, tag="ycn", name="ycn")
                    for mi in range(2):
                        nc.sync.dma_start(ycn[:, mi, :],
                                          d_yc[96 * mi:96 * (mi + 1), c0:c0 + 512])
                    # t1f = gamma*t1 = ycn(= g*a*y) + tp * (gamma/256)
                    t1f = sbc.tile([96, 2, 512], BF16, tag="t1f", name="t1f")
                    for mi in range(2):
                        nc.vector.scalar_tensor_tensor(
                            out=t1f[:, mi, :], in0=tp[mi][:96, :],
                            scalar=gscv[0:96, :],
                            in1=ycn[:, mi, :], op0=OP.mult, op1=OP.add)
                    t1c8 = sbc.tile([96, 512, 2], FP8, tag="t1c8", name="t1c8")
                    nc.vector.tensor_copy(
                        t1c8[:].rearrange("p n two -> p two n"), t1f[:])
                    t1_q[ck] = (t1f, t1c8)

                def stage_b(ck):
                    t1f, t1c8 = t1_q.pop(ck)
                    c0 = ck * 512
                    # ffn1 + gelu: fp1 = 8x f1_true; gc8 = gelu(f1_true)
                    gc8 = [sbg.tile([128, 512, 2], FP8, tag="gc8", name="gc8")
                           for _ in range(3)]
                    for mt in range(6):
                        fp1 = pcv.tile([128, 512], F32, tag="cv", name="cv")
                        nc.tensor.matmul(fp1[:],
                                         wf1_t[:, :, 128 * mt:128 * (mt + 1)],
                                         t1c8[:].rearrange("p n two -> p two n"),
                                         start=True, stop=True,
                                         perf_mode=DRM)
                        nc.scalar.activation(gc8[mt // 2][:, :, mt % 2], fp1[:],
                                             AF.Gelu, scale=0.125)
                    # ffn2: fp2 = 8*delta*f2_true; t2 = t1f + fp2/8
                    t2c = sbc.tile([96, 2, 512], BF16, tag="t2c", name="t2c")
                    for mi in range(2):
                        fp2 = pcv.tile([128, 512], F32, tag="cv", name="cv")
                        for jp in range(3):
                            nc.tensor.matmul(fp2[:96, :],
                                             wf2_t[:, jp, :, 96 * mi:96 * (mi + 1)],
                                             gc8[jp][:].rearrange(
                                                 "p n two -> p two n"),
                                             start=(jp == 0),
                                             stop=(jp == 2), perf_mode=DRM)
                        nc.vector.scalar_tensor_tensor(
                            out=t2c[:, mi, :], in0=fp2[:96, :], scalar=0.125,
                            in1=t1f[:, mi, :], op0=OP.mult, op1=OP.add)
                    # proj (bf16): accumulate over the two 96-ch planes
                    for mi, (mo, ms) in enumerate(KB):
                        pp = pcv.tile([128, 512], F32, tag="cv", name="cv")
                        for pl in range(2):
                            nc.tensor.matmul(pp[:ms, :],
                                             wpr_t[:, pl, mo:mo + ms],
                                             t2c[:, pl, :],
                                             start=(pl == 0), stop=(pl == 1))
                        oc = sbc.tile([128, 512], F32, tag=f"oc{mi}", name=f"oc{mi}")
                        if mi == 0:
                            nc.scalar.copy(oc[:ms, :], pp[:ms, :])
                        else:
                            nc.vector.tensor_copy(oc[:ms, :], pp[:ms, :])
                        nc.sync.dma_start(d_out[mo:mo + ms, c0:c0 + 512],
                                          oc[:ms, :])

                if STAGE >= 4:
                    stage_a(0)
                    stage_t1(0)
                    for ck in range(NCK):
                        if ck + 1 < NCK:
                            stage_a(ck + 1)
                        stage_b(ck)
                        if ck + 1 < NCK:
                            stage_t1(ck + 1)

    nc.compile()
    return nc


_NC = None


def _get_nc():
    global _NC
    if _NC is None:
        _NC = build_nc()
    return _NC


def _prep_weights(q_w, q_dw_w, kv_w, kv_dw_w, linear_w, proj_w, ffn1_w, ffn2_w,
                  temperature, alpha, beta, gamma, delta):
    def pad_oc(w):  # [192 real oc, ic] -> [ic, 256 padded oc]
        out = np.zeros((C, CP), np.float32)
        for h in range(HEADS):
            out[:, CPH * h:CPH * h + CH] = w[CH * h:CH * (h + 1), :].T
        return out

    wq = pad_oc(np.asarray(q_w, np.float32)) * 8.0
    kv = np.asarray(kv_w, np.float32)
    wkv = np.concatenate([pad_oc(kv[:C]), pad_oc(kv[C:])], axis=1) * 8.0

    # [192,1,3,3] -> [256, 9, 128] diag, slots = DW_PAIRS order + center
    slot_tap = [0, 2, 3, 5, 6, 8, 1, 7, 4]

    def pad_dw(w):
        out = np.zeros((CP, 9, 128), np.float32)
        for h in range(HEADS):
            for j in range(CH):
                cp = CPH * h + j
                taps = w[CH * h + j, 0].reshape(9)
                for s, t in enumerate(slot_tap):
                    out[cp, s, cp % 128] = taps[t]
        return out * 32.0

    qdw = pad_dw(np.asarray(q_dw_w, np.float32))
    kvd = np.asarray(kv_dw_w, np.float32)
    kvdw = np.concatenate([pad_dw(kvd[:C]), pad_dw(kvd[C:])], axis=0)

    gamma_f = float(gamma)
    # wlin (bf16) = beta*lin padded; W_comb = attn x wlin -> tp = 256x t'_true
    lin = np.asarray(linear_w, np.float32) * float(beta)
    wlin = np.zeros((CP, C), np.float32)
    for h in range(HEADS):
        wlin[CPH * h:CPH * h + CH, :] = lin[:, CH * h:CH * (h + 1)].T
    wlin8 = wlin.reshape(2, 128, C).transpose(1, 0, 2)

    # t1c8 = gamma*t1; wf18 = ffn1_w^T * 8/gamma -> fp1 = 8x f1_true
    wf1 = np.asarray(ffn1_w, np.float32).T * (8.0 / gamma_f)
    wf18 = wf1.reshape(2, 96, 768).transpose(1, 0, 2)
    # wf28 = ffn2_w^T * 8*delta -> fp2 = 8*delta*f2_true
    wf2 = np.asarray(ffn2_w, np.float32).T * (8.0 * float(delta))
    wf28 = wf2.reshape(3, 2, 128, C).transpose(2, 0, 1, 3)
    wpr = np.asarray(proj_w, np.float32).T.reshape(2, 96, C).transpose(1, 0, 2)

    tempb = np.zeros((128, 2), np.float32)
    tv = np.asarray(temperature, np.float32).reshape(HEADS)
    for h in range(HEADS):
        tempb[64 * (h % 2):64 * (h % 2) + 64, h // 2] = tv[h]

    gscv = np.full((128, 1), gamma_f / 256.0, np.float32)
    id128 = np.eye(128, dtype=np.float32)

    return {
        "_yscale": gamma_f * float(alpha),
        "wq": wq.reshape(2, 96, CP).transpose(1, 0, 2).astype(f8),
        "wkv": wkv.reshape(2, 96, 2 * CP).transpose(1, 0, 2).astype(f8),
        "qdw": qdw.astype(f8), "kvdw": kvdw.astype(f8),
        "wlin": wlin8.astype(bf16).copy(), "wf1": wf18.astype(f8),
        "wf2": wf28.astype(f8), "wpr": wpr.astype(bf16).copy(),
        "tempb": tempb, "gamma": gscv,
        "id128": id128,
    }


def _make_in_maps(x, y, shared):
    shared = dict(shared)
    yscale = shared.pop("_yscale")
    in_maps = []
    for c in range(N_CORES):
        bi, s = c // 2, c % 2
        r0 = s * HLOC
        xe = np.zeros((C, ER, EC), np.float32)
        ye = np.zeros((C, ER, EC), np.float32)
        rlo, rhi = max(r0 - 1, 0), min(r0 + HLOC + 1, H)
        elo = rlo - (r0 - 1)
        xe[:, elo:elo + (rhi - rlo), 1:129] = x[bi, :, rlo:rhi, :]
        ye[:, elo:elo + (rhi - rlo), 1:129] = y[bi, :, rlo:rhi, :]
        m = dict(shared)
        m["xe"] = xe.reshape(2, 96, NEXT).transpose(1, 2, 0).astype(f8)
        m["ye"] = ye.reshape(2, 96, NEXT).transpose(1, 2, 0).astype(f8)
        m["yc"] = (yscale * y[bi, :, r0:r0 + HLOC, :]
                   ).reshape(C, NLOC).astype(bf16)
        in_maps.append(m)
    return in_maps


def kernel(**inputs):
    x = np.asarray(inputs["x"], np.float32)
    y = np.asarray(inputs["y"], np.float32)
    shared = _prep_weights(
        inputs["q_w"], inputs["q_dw_w"], inputs["kv_w"], inputs["kv_dw_w"],
        inputs["linear_w"], inputs["proj_w"], inputs["ffn1_w"], inputs["ffn2_w"],
        inputs["temperature"], inputs["alpha"], inputs["beta"],
        inputs["gamma"], inputs["delta"])

    in_maps = _make_in_maps(x, y, shared)

    nc = _get_nc()
    res = run_bass_kernel_spmd(nc, in_maps, list(range(N_CORES)))
    out = np.empty((B, C, H, W), np.float32)
    for c in range(N_CORES):
        bi, s = c // 2, c % 2
        out[bi, :, s * HLOC:(s + 1) * HLOC, :] = \
            res.results[c]["out"].reshape(C, HLOC, W)
    return out



# revision 74
# speedup vs baseline: 1.1887x; 1.0425x over previous
"""Trainium2 Bass kernel for nn_CDEM_62079457296798 (channel-attention
transformer block).

Sharding: 8 cores = 4 batches x 2 spatial halves (64 rows + 1 halo row each).
Cross-core communication: two small per-band-pair AllReduces carrying the
channel-attention Gram matrices and q/k l2-norm sums; everything else local.

Layout: channel-major activations [C_part, pixels_free]; attention channels
padded 48 -> 64 per head. Heavy use of fp8e4m3 DoubleRow matmuls (2 K-planes
per instruction; planes interleaved in memory so the PE streams 2 rows/cycle):
the q/kv 1x1 convs pair the 192 input channels as [96, 2]; the depthwise 3x3
runs as 4 tap-pair DoubleRow matmuls with diagonal [128, 2, 128] weights
(overlapping-stride pair APs over the padded image) + 1 bf16-free center tap;
ffn1/ffn2 pair K the same way. q/k sq-norms ride the Gram matmuls
(qg = q^T [q|k], kg = k^T k; diag extracted via masked reduce). z and lin are
fused: W_comb = beta*lin*attn is built once after softmax, so the per-chunk
trunk is W_comb^T v -> t1 -> ffn -> proj (proj in bf16). All runtime scalars
(alpha/beta/gamma/delta) are folded host-side; kernel-side rescales are
compile-time powers of two. The trunk is software-pipelined (tp one chunk
ahead) and interleaved with v-band production to keep the PE stream dense.
"""
import sys
sys.path.insert(0, '/opt/trn_rl_repo')

import numpy as np
import ml_dtypes

import bass_rust
from concourse import bacc, mybir, tile
from concourse.bass import _add_dep_helper
from concourse.bass_utils import run_bass_kernel_spmd

F32 = mybir.dt.float32
F32R = mybir.dt.float32r
BF16 = mybir.dt.bfloat16
FP8 = mybir.dt.float8e4
DRM = mybir.MatmulPerfMode.DoubleRow
AF = mybir.ActivationFunctionType
OP = mybir.AluOpType
bf16 = ml_dtypes.bfloat16
f8 = ml_dtypes.float8_e4m3fn

# depthwise 3x3 as 4 fp8 DoubleRow pairs + 1 single (tap index t = 3*(dr+1)+(dc+1));
# pair strides in elements of the [ER, EC] image (2 = two cols, 260 = two rows)
DW_PAIRS = [(0, 2, 2), (3, 5, 2), (6, 8, 2), (1, 7, 2 * 130)]
DW_SINGLE = 4


def _pair_ap(base, stride):
    raw = base.ap.copy()
    return bass_rust.AP(base.tensor, base.offset,
                        [raw[0], [stride, 2]] + list(raw[1:]))

N_CORES = 8
B, C, H, W = 4, 192, 128, 128
HEADS, CH = 4, 48
CPH = 64                # padded channels per head
CP = HEADS * CPH        # 256 padded attn channels
HLOC = 64               # image rows per core
ER, EC = 66, 130        # ext rows/cols (halo + zero pad)
NEXT = ER * EC          # 8580
NLOC = HLOC * W         # 8192
NCK = 16                # output chunks (4 rows x 128 = 512 px)
CONV_CHUNKS = [(i * 512, 512) for i in range(16)] + [(16 * 512, NEXT - 16 * 512)]
GRP = 2048
CONV_GROUPS = [(i * GRP, GRP) for i in range(4)] + [(4 * GRP, NEXT - 4 * GRP)]
KB = [(0, 128), (128, 64)]          # 192-channel K bands

DIRECT_PSUM_OUT = False  # DMA cannot read PSUM on TRN2


import os
STAGE = int(os.environ.get("KSTAGE", "4"))
KSUB = int(os.environ.get("KSUB", "4"))


class _StageDone(Exception):
    pass


def build_nc():
    nc = bacc.Bacc("TRN2", target_bir_lowering=False, debug=False,
                   num_devices=N_CORES)

    d_xe = nc.dram_tensor("xe", [96, NEXT, 2], FP8, kind="ExternalInput")
    d_ye = nc.dram_tensor("ye", [96, NEXT, 2], FP8, kind="ExternalInput")
    d_yc = nc.dram_tensor("yc", [C, NLOC], BF16, kind="ExternalInput")
    d_wq = nc.dram_tensor("wq", [96, 2, CP], FP8, kind="ExternalInput")
    d_wkv = nc.dram_tensor("wkv", [96, 2, 2 * CP], FP8, kind="ExternalInput")
    d_qdw = nc.dram_tensor("qdw", [CP, 9, 128], FP8, kind="ExternalInput")
    d_kvdw = nc.dram_tensor("kvdw", [2 * CP, 9, 128], FP8, kind="ExternalInput")
    d_wlin = nc.dram_tensor("wlin", [128, 2, C], BF16, kind="ExternalInput")
    d_wf1 = nc.dram_tensor("wf1", [96, 2, 768], FP8, kind="ExternalInput")
    d_wf2 = nc.dram_tensor("wf2", [128, 3, 2, C], FP8, kind="ExternalInput")
    d_wpr = nc.dram_tensor("wpr", [96, 2, C], BF16, kind="ExternalInput")
    d_tempb = nc.dram_tensor("tempb", [128, 2], F32, kind="ExternalInput")
    d_gamma = nc.dram_tensor("gamma", [128, 1], F32, kind="ExternalInput")
    d_id128 = nc.dram_tensor("id128", [128, 128], F32, kind="ExternalInput")
    d_out = nc.dram_tensor("out", [C, NLOC], F32, kind="ExternalOutput")
    cc_in = nc.dram_tensor("cc_in", [112, 228], F32)
    cc_out = nc.dram_tensor("cc_out", [112, 228], F32)

    with tile.TileContext(nc) as tc:
        with (
            tc.tile_pool(name="sbw", bufs=1) as sbw,      # weights/consts
            tc.tile_pool(name="sbpre", bufs=2) as sbpre,  # conv1x1 out (ext img)
            tc.tile_pool(name="sbin", bufs=5) as sbin,    # streamed conv inputs
            tc.tile_pool(name="sbqk", bufs=4) as sbqk,    # q/k chunk tiles
            tc.tile_pool(name="sbT", bufs=1) as sbT,      # qT/kT/v persistents
            tc.tile_pool(name="sbs", bufs=1) as sbs,      # small attn tiles
            tc.tile_pool(name="sbc", bufs=3) as sbc,      # trunk chunk pipeline
            tc.tile_pool(name="sbg", bufs=6) as sbg,      # gelu chunk tiles
            tc.tile_pool(name="pcv", bufs=3, space="PSUM") as pcv,
            tc.tile_pool(name="pdw", bufs=2, space="PSUM") as pdw,
            tc.tile_pool(name="psm", bufs=1, space="PSUM") as psm,
        ):
            # ---------- weights ----------
            wq_t = sbw.tile([96, 2, CP], FP8, tag="wq", name="wq")
            wkv_t = sbw.tile([96, 2, 2 * CP], FP8, tag="wkv", name="wkv")
            nc.sync.dma_start(wq_t[:], d_wq.ap())
            qdw_t = [sbw.tile([128, 9, 128], FP8, tag=f"qdw{m}", name=f"qdw{m}") for m in range(2)]
            kvdw_t = [sbw.tile([128, 9, 128], FP8, tag=f"kvdw{m}", name=f"kvdw{m}") for m in range(4)]
            id128 = sbw.tile([128, 128], F32, tag="id128", name="id128")
            for m in range(2):
                nc.sync.dma_start(qdw_t[m][:], d_qdw[128 * m:128 * (m + 1)])
            nc.sync.dma_start(id128[:], d_id128.ap())

            def load_kv_weights():
                nc.sync.dma_start(wkv_t[:], d_wkv.ap())
                for m in range(4):
                    nc.sync.dma_start(kvdw_t[m][:], d_kvdw[128 * m:128 * (m + 1)])
            wlin_t = sbw.tile([128, 2, C], BF16, tag="wlin", name="wlin")
            wf1_t = sbw.tile([96, 2, 768], FP8, tag="wf1", name="wf1")
            wf2_t = sbw.tile([128, 3, 2, C], FP8, tag="wf2", name="wf2")
            wpr_t = sbw.tile([96, 2, C], BF16, tag="wpr", name="wpr")
            tempb = sbw.tile([128, 2], F32, tag="tempb", name="tempb")
            gscv = sbw.tile([128, 1], F32, tag="gscv", name="gscv")

            def load_trunk_weights():
                nc.sync.dma_start(wlin_t[:], d_wlin.ap())
                nc.sync.dma_start(wf1_t[:], d_wf1.ap())
                nc.sync.dma_start(wf2_t[:], d_wf2.ap())
                nc.sync.dma_start(wpr_t[:], d_wpr.ap())
                nc.sync.dma_start(tempb[:], d_tempb.ap())
                nc.sync.dma_start(gscv[:], d_gamma.ap())

            # persistent attn-path results; qkT packs q (cols 0:112) and
            # k (cols 112:224) transposed per band-pair
            qkT = [sbT.tile([128, 64, 224], BF16, tag=f"qkT{p}", name=f"qkT{p}")
                   for p in range(2)]
            vband = [sbT.tile([128, NLOC], BF16, tag=f"v{m}", name=f"v{m}") for m in range(2)]
            # gram + norm accumulators live in the tp-tag PSUM (idle pre-trunk)
            qgacc = pcv.tile([112, 448], F32, tag="tp", bufs=2, name="qgacc")
            kgacc = pcv.tile([112, 224], F32, tag="tp", bufs=2, name="kgacc")

            # ============ q/k/v production ============
            def band_producer(src_dram, w_t, dw_tiles, m, sink,
                              collect_mms=None):
                """One 128-wide band: conv1x1 (fp8 DR) + depthwise 3x3.
                Returns (need_groups, emit_dw) for interleaved emission."""
                pre = sbpre.tile([128, ER, EC], FP8, tag="pre", name="pre")
                pref = pre[:].rearrange("p a b -> p (a b)")
                state = {"g": 0, "ci": 0}

                def need_groups(ng):
                    while state["g"] < min(ng, len(CONV_GROUPS)):
                        g0, gn = CONV_GROUPS[state["g"]]
                        xc = sbin.tile([96, GRP, 2], FP8, tag="xin", name="xin")
                        nc.sync.dma_start(xc[:, :gn, :],
                                          src_dram[:, g0:g0 + gn, :])
                        for c0 in range(0, gn, 512):
                            cn = min(512, gn - c0)
                            ps = pcv.tile([128, 512], F32, tag="cv", name="cv")
                            mm = nc.tensor.matmul(
                                ps[:, :cn],
                                w_t[:, :, 128 * m:128 * (m + 1)],
                                xc[:, c0:c0 + cn, :].rearrange(
                                    "p n two -> p two n"),
                                start=True, stop=True, perf_mode=DRM)
                            if collect_mms is not None:
                                collect_mms.append(mm)
                            if state["ci"] % 2 == 0:
                                nc.vector.tensor_copy(
                                    pref[:, g0 + c0:g0 + c0 + cn], ps[:, :cn])
                            else:
                                nc.scalar.copy(
                                    pref[:, g0 + c0:g0 + c0 + cn], ps[:, :cn])
                            state["ci"] += 1
                        state["g"] += 1

                def emit_dw(ck):
                    r0 = 1 + 4 * ck
                    dp = pdw.tile([128, 4, 128], F32, tag="dw", name="dw")
                    for i, (ta, tb, stride) in enumerate(DW_PAIRS):
                        dra, dca = ta // 3 - 1, ta % 3 - 1
                        base = pre[:, r0 + dra:r0 + 4 + dra,
                                   1 + dca:129 + dca]
                        nc.tensor.matmul(
                            dp[:], dw_tiles[m][:, 2 * i:2 * i + 2, :],
                            _pair_ap(base, stride),
                            start=(i == 0), stop=False, perf_mode=DRM)
                    nc.tensor.matmul(
                        dp[:], dw_tiles[m][:, 8, :],
                        pre[:, r0:r0 + 4, 1:129],
                        start=False, stop=True)
                    sink(m, ck, dp[:].rearrange("p a b -> p (a b)"))

                return need_groups, emit_dw

            def conv_dw_path(src_dram, w_t, dw_tiles, n_mb, sink, m_off=0,
                             collect_mms=None):
                for m in range(m_off, m_off + n_mb):
                    ng, edw = band_producer(src_dram, w_t, dw_tiles, m, sink,
                                            collect_mms)
                    ng(len(CONV_GROUPS))
                    for ck in range(NCK if KSUB >= 2 else 0):
                        edw(ck)

            def qk_sink(coff):
                qcbig = [None]

                def sink(m, ck, flat):
                    j = ck % 4
                    if j == 0:
                        qcbig[0] = sbqk.tile([128, 2048], BF16, tag="qkc", name="qkc")
                    qc = qcbig[0][:, 512 * j:512 * (j + 1)]
                    if ck % 2 == 0:
                        nc.vector.tensor_copy(qc, flat)
                    else:
                        nc.scalar.copy(qc, flat)
                    if KSUB >= 4 and j == 3:
                        nc.sync.dma_start_transpose(
                            qkT[m][:, 4 * ck - 12:4 * ck + 4, coff:coff + 112],
                            qcbig[0][0:112, :])
                return sink

            def v_sink(m, ck, flat):
                dst = vband[m - 2]
                if ck % 2 == 0:
                    nc.vector.tensor_copy(dst[:, ck * 512:(ck + 1) * 512], flat)
                else:
                    nc.scalar.copy(dst[:, ck * 512:(ck + 1) * 512], flat)

            sinkq = qk_sink(0)
            sinkk = qk_sink(112)
            sqv = sbs.tile([128, 2], F32, tag="sqv", name="sqv")
            skv = sbs.tile([128, 2], F32, tag="skv", name="skv")

            def run_band(src, w_t, dwt, m, sink):
                ng, edw = band_producer(src, w_t, dwt, m, sink)
                ng(len(CONV_GROUPS))
                for ck in range(NCK if KSUB >= 2 else 0):
                    edw(ck)

            def gram_chunks(p, ck0, ck1):
                # gram + q/k sq-norms for band-pair p (qg: [q^T q | q^T k],
                # kg: k^T k)
                for ck in range(ck0, ck1):
                    nc.tensor.matmul(qgacc[:, 224 * p:224 * (p + 1)],
                                     qkT[p][:, ck, 0:112], qkT[p][:, ck, :],
                                     start=(ck == 0), stop=(ck == 63))
                    nc.tensor.matmul(kgacc[:, 112 * p:112 * (p + 1)],
                                     qkT[p][:, ck, 112:224],
                                     qkT[p][:, ck, 112:224],
                                     start=(ck == 0), stop=(ck == 63))

            def finish_ar():
                gsb = sbs.tile([112, 228], F32, tag="gsb", name="gsb")
                for p in range(2):
                    dsc = sbs.tile([112, 112], F32, tag=f"dsc{p}", name=f"dsc{p}")
                    nc.vector.tensor_tensor(
                        out=dsc[:], in0=qgacc[:, 224 * p:224 * p + 112],
                        in1=id128[0:112, 0:112], op=OP.mult)
                    nc.vector.tensor_reduce(sqv[0:112, p:p + 1], dsc[:],
                                            axis=mybir.AxisListType.X, op=OP.add)
                    dsc2 = sbs.tile([112, 112], F32, tag=f"dsc2{p}", name=f"dsc2{p}")
                    nc.vector.tensor_tensor(
                        out=dsc2[:], in0=kgacc[:, 112 * p:112 * (p + 1)],
                        in1=id128[0:112, 0:112], op=OP.mult)
                    nc.vector.tensor_reduce(skv[0:112, p:p + 1], dsc2[:],
                                            axis=mybir.AxisListType.X, op=OP.add)
                    nc.vector.tensor_copy(gsb[:, 112 * p:112 * (p + 1)],
                                          qgacc[:, 224 * p + 112:224 * (p + 1)])
                nc.vector.tensor_copy(gsb[:, 224:226], sqv[0:112, :])
                nc.vector.tensor_copy(gsb[:, 226:228], skv[0:112, :])
                nc.scalar.dma_start(cc_in.ap()[:, :], gsb[:, :])
                nc.gpsimd.collective_compute(
                    "AllReduce", OP.add,
                    replica_groups=[[0, 1], [2, 3], [4, 5], [6, 7]],
                    ins=[cc_in.ap()], outs=[cc_out.ap()])

            run_band(d_xe, wq_t, qdw_t, 0, sinkq)
            load_kv_weights()
            run_band(d_ye, wkv_t, kvdw_t, 0, sinkk)
            # q band 1 with gram p0 interleaved (needs only q0/k0)
            ngq1, edwq1 = band_producer(d_xe, wq_t, qdw_t, 1, sinkq)
            ngq1(len(CONV_GROUPS))
            for ck in range(NCK if KSUB >= 2 else 0):
                edwq1(ck)
                if STAGE >= 3:
                    gram_chunks(0, 4 * ck, 4 * ck + 4)
            # k band 1 with gram p1 lag-interleaved (4 chunks behind the
            # transposes feeding qkT[1])
            ngk1, edwk1 = band_producer(d_ye, wkv_t, kvdw_t, 1, sinkk)
            ngk1(len(CONV_GROUPS))
            for ck in range(NCK if KSUB >= 2 else 0):
                edwk1(ck)
                if STAGE >= 3 and ck >= 4:
                    gram_chunks(1, 4 * (ck - 4), 4 * (ck - 4) + 4)
            load_trunk_weights()

            if STAGE < 3:
                oc0 = sbs.tile([128, 2], F32, tag="oc0d", name="oc0d")
                nc.vector.tensor_copy(oc0[:], sqv[:])
                nc.sync.dma_start(d_out[0:128, 0:2], oc0[:])
            if STAGE >= 3:
                # prefetch ALL of v2's conv inputs, then dispatch the single
                # AllReduce: v2's DMA-free dw work covers the wire phase,
                # which blocks input DMAs for ~15us
                v_mms = []
                ngv2, edwv2 = band_producer(d_ye, wkv_t, kvdw_t, 2, v_sink,
                                            collect_mms=v_mms)
                ngv2(len(CONV_GROUPS))
                gram_chunks(1, 48, 64)
                finish_ar()
                for ck in range(NCK):
                    edwv2(ck)
                vng3, vdw3 = band_producer(d_ye, wkv_t, kvdw_t, 3, v_sink,
                                           collect_mms=v_mms)
                vng3(len(CONV_GROUPS))
                for ck in range(6):
                    vdw3(ck)

                gg = sbs.tile([112, 224], F32, tag="gg", name="gg")
                sqg = sbs.tile([128, 2], F32, tag="sqg", name="sqg")
                skg = sbs.tile([128, 2], F32, tag="skg", name="skg")
                nc.vector.memset(sqg[:], 1.0)
                nc.vector.memset(skg[:], 1.0)
                nc.sync.dma_start(gg[:], cc_out.ap()[:, 0:224])
                nc.sync.dma_start(sqg[0:112, :], cc_out.ap()[:, 224:226])
                nc.sync.dma_start(skg[0:112, :], cc_out.ap()[:, 226:228])

                # ============ attention finalize ============
                def rsqrt_newton(tag, s_t):
                    sc = sbs.tile([128, 2], F32, tag=tag + "_c")
                    nc.vector.tensor_scalar_max(sc[:], s_t[:], 1e-24)
                    rt = sbs.tile([128, 2], F32, tag=tag + "_s")
                    nc.scalar.activation(rt[:], sc[:], AF.Sqrt)
                    r0 = sbs.tile([128, 2], F32, tag=tag + "_r0")
                    nc.vector.reciprocal(r0[:], rt[:])
                    rr = sbs.tile([128, 2], F32, tag=tag + "_rr")
                    nc.vector.tensor_tensor(out=rr[:], in0=r0[:], in1=r0[:], op=OP.mult)
                    t1_ = sbs.tile([128, 2], F32, tag=tag + "_t1")
                    nc.vector.scalar_tensor_tensor(out=t1_[:], in0=sc[:], scalar=-0.5,
                                                   in1=rr[:], op0=OP.mult, op1=OP.mult)
                    nc.vector.tensor_scalar_add(t1_[:], t1_[:], 1.5)
                    rv = sbs.tile([128, 2], F32, tag=tag)
                    nc.vector.tensor_tensor(out=rv[:], in0=r0[:], in1=t1_[:], op=OP.mult)
                    return rv

                rq = rsqrt_newton("rq", sqg)
                rk = rsqrt_newton("rk", skg)
                srow = sbs.tile([128, 2], F32, tag="srow", name="srow")
                nc.vector.tensor_tensor(out=srow[:], in0=rq[:], in1=tempb[:], op=OP.mult)

                srow_r, scol_r = [], []
                for p in range(2):
                    for src, lst, nm in ((srow, srow_r, "sr"), (rk, scol_r, "sc")):
                        fp = psm.tile([1, 112], F32, tag="sm", name="sm")
                        nc.tensor.transpose(fp[:], src[0:112, p:p + 1],
                                            id128[0:112, 0:112])
                        fr = sbs.tile([1, 112], F32R, tag=f"{nm}{p}", name=f"{nm}{p}")
                        nc.vector.tensor_copy(fr[:], fp[:])
                        lst.append(fr)

                # W_comb = beta*lin*attn, [v-ch(pad 128), plane p, out 192];
                # pad v rows stay zero (vband pad rows are zero anyway)
                wcs = sbs.tile([128, 2, C], BF16, tag="wcs", name="wcs")
                nc.gpsimd.memset(wcs[:], 0.0)
                for p in range(2):
                    spair = psm.tile([112, 112], F32, tag="sm", name="sm")
                    nc.tensor.matmul(spair[:], srow_r[p][:], scol_r[p][:],
                                     start=True, stop=True)
                    lg = sbs.tile([112, 112], F32, tag="lg", name="lg")
                    nc.vector.tensor_tensor(out=lg[:], in0=gg[:, 112 * p:112 * (p + 1)],
                                            in1=spair[:], op=OP.mult)
                    at16 = sbs.tile([112, 112], BF16, tag="at16", name="at16")
                    for e in range(2):
                        sl = slice(64 * e, 64 * e + 48)
                        mx = sbs.tile([112, 1], F32, tag="mx", name="mx")
                        nc.vector.tensor_reduce(mx[sl, :], lg[sl, sl],
                                                axis=mybir.AxisListType.X, op=OP.max)
                        exh = sbs.tile([112, 112], F32, tag="exh", name="exh")
                        nc.vector.tensor_scalar(out=exh[sl, 0:48], in0=lg[sl, sl],
                                                scalar1=mx[sl, :], scalar2=None,
                                                op0=OP.subtract)
                        ex2 = sbs.tile([112, 112], F32, tag="ex2", name="ex2")
                        den = sbs.tile([112, 1], F32, tag="den", name="den")
                        nc.scalar.activation(ex2[sl, 0:48], exh[sl, 0:48], AF.Exp,
                                             accum_out=den[sl, :])
                        rc0 = sbs.tile([112, 1], F32, tag="rc0", name="rc0")
                        nc.vector.reciprocal(rc0[sl, :], den[sl, :])
                        nt = sbs.tile([112, 1], F32, tag="nt", name="nt")
                        nc.vector.tensor_tensor(out=nt[sl, :], in0=den[sl, :],
                                                in1=rc0[sl, :], op=OP.mult)
                        nc.vector.tensor_scalar(out=nt[sl, :], in0=nt[sl, :],
                                                scalar1=-1.0, scalar2=2.0,
                                                op0=OP.mult, op1=OP.add)
                        rc1 = sbs.tile([112, 1], F32, tag="rc1", name="rc1")
                        nc.vector.tensor_tensor(out=rc1[sl, :], in0=rc0[sl, :],
                                                in1=nt[sl, :], op=OP.mult)
                        nc.vector.tensor_scalar(out=at16[sl, 0:48], in0=ex2[sl, 0:48],
                                                scalar1=rc1[sl, :], scalar2=None,
                                                op0=OP.mult)
                        wcp = pcv.tile([128, 512], F32, tag="cv", name="cv")
                        nc.tensor.matmul(wcp[sl, :C], at16[sl, 0:48],
                                         wlin_t[64 * e:64 * e + 48, p, :],
                                         start=True, stop=True)
                        nc.vector.tensor_copy(wcs[sl, p, :], wcp[sl, :C])

                if STAGE < 4:
                    for ck in range(6, NCK):
                        vdw3(ck)
                    ocx = sbs.tile([112, 64], F32, tag="ocx", name="ocx")
                    nc.vector.tensor_copy(ocx[:], wcs[0:112, 0, 0:64])
                    nc.sync.dma_start(d_out[0:112, 0:64], ocx[:])
                # ==== software-pipelined trunk, interleaved with v band-3 ====
                # stage A(ck): v-dw chunk (6 ahead) + tp matmuls — one iter ahead
                # stage B(ck): t1 -> ffn1 -> gelu -> ffn2 -> t2 -> proj
                tp_q, t1_q = {}, {}

                def stage_a(ck):
                    if ck + 6 < NCK:
                        vdw3(ck + 6)
                    c0 = ck * 512
                    # fused z+lin: tp = W_comb^T v = 256x t'_true
                    tp = [pcv.tile([128, 512], F32, tag="tp", bufs=2,
                                   name="tp") for _ in range(2)]
                    for mi in range(2):
                        for p in range(2):
                            nc.tensor.matmul(tp[mi][:96, :],
                                             wcs[:, p, 96 * mi:96 * (mi + 1)],
                                             vband[p][:, c0:c0 + 512],
                                             start=(p == 0), stop=(p == 1))
                    tp_q[ck] = tp

                def stage_t1(ck):
                    tp = tp_q.pop(ck)
                    c0 = ck * 512
                    ycn = sbc.tile([96, 2, 512], BF16, tag="ycn", name="ycn")
                    for mi in range(2):
                        nc.sync.dma_start(ycn[:, mi, :],
                                          d_yc[96 * mi:96 * (mi + 1), c0:c0 + 512])
                    # t1f = gamma*t1 = ycn(= g*a*y) + tp * (gamma/256)
                    t1f = sbc.tile([96, 2, 512], BF16, tag="t1f", name="t1f")
                    for mi in range(2):
                        nc.vector.scalar_tensor_tensor(
                            out=t1f[:, mi, :], in0=tp[mi][:96, :],
                            scalar=gscv[0:96, :],
                            in1=ycn[:, mi, :], op0=OP.mult, op1=OP.add)
                    t1c8 = sbc.tile([96, 512, 2], FP8, tag="t1c8", name="t1c8")
                    nc.vector.tensor_copy(
                        t1c8[:].rearrange("p n two -> p two n"), t1f[:])
                    t1_q[ck] = (t1f, t1c8)

                def stage_b(ck):
                    t1f, t1c8 = t1_q.pop(ck)
                    c0 = ck * 512
                    # ffn1 + gelu: fp1 = 8x f1_true; gc8 = gelu(f1_true)
                    gc8 = [sbg.tile([128, 512, 2], FP8, tag="gc8", name="gc8")
                           for _ in range(3)]
                    for mt in range(6):
                        fp1 = pcv.tile([128, 512], F32, tag="cv", name="cv")
                        nc.tensor.matmul(fp1[:],
                                         wf1_t[:, :, 128 * mt:128 * (mt + 1)],
                                         t1c8[:].rearrange("p n two -> p two n"),
                                         start=True, stop=True,
                                         perf_mode=DRM)
                        nc.scalar.activation(gc8[mt // 2][:, :, mt % 2], fp1[:],
                                             AF.Gelu, scale=0.125)
                    # ffn2: fp2 = 8*delta*f2_true; t2 = t1f + fp2/8
                    t2c = sbc.tile([96, 2, 512], BF16, tag="t2c", name="t2c")
                    for mi in range(2):
                        fp2 = pcv.tile([128, 512], F32, tag="cv", name="cv")
                        for jp in range(3):
                            nc.tensor.matmul(fp2[:96, :],
                                             wf2_t[:, jp, :, 96 * mi:96 * (mi + 1)],
                                             gc8[jp][:].rearrange(
                                                 "p n two -> p two n"),
                                             start=(jp == 0),
                                             stop=(jp == 2), perf_mode=DRM)
                        nc.vector.scalar_tensor_tensor(
                            out=t2c[:, mi, :], in0=fp2[:96, :], scalar=0.125,
                            in1=t1f[:, mi, :], op0=OP.mult, op1=OP.add)
                    # proj (bf16): accumulate over the two 96-ch planes
                    for mi, (mo, ms) in enumerate(KB):
                        pp = pcv.tile([128, 512], F32, tag="cv", name="cv")
                        for pl in range(2):
                            nc.tensor.matmul(pp[:ms, :],
                                             wpr_t[:, pl, mo:mo + ms],
                                             t2c[:, pl, :],
                                             start=(pl == 0), stop=(pl == 1))
                        oc = sbc.tile([128, 512], F32, tag=f"oc{mi}", name=f"oc{mi}")
                        if mi == 0:
                            nc.scalar.copy(oc[:ms, :], pp[:ms, :])
                        else:
                            nc.vector.tensor_copy(oc[:ms, :], pp[:ms, :])
                        nc.sync.dma_start(d_out[mo:mo + ms, c0:c0 + 512],
                                          oc[:ms, :])

                if STAGE >= 4:
                    stage_a(0)
                    stage_t1(0)
                    for ck in range(NCK):
                        if ck + 1 < NCK:
                            stage_a(ck + 1)
                        stage_b(ck)
                        if ck + 1 < NCK:
                            stage_t1(ck + 1)

    nc.compile()
    return nc


_NC = None


def _get_nc():
    global _NC
    if _NC is None:
        _NC = build_nc()
    return _NC


def _prep_weights(q_w, q_dw_w, kv_w, kv_dw_w, linear_w, proj_w, ffn1_w, ffn2_w,
                  temperature, alpha, beta, gamma, delta):
    def pad_oc(w):  # [192 real oc, ic] -> [ic, 256 padded oc]
        out = np.zeros((C, CP), np.float32)
        for h in range(HEADS):
            out[:, CPH * h:CPH * h + CH] = w[CH * h:CH * (h + 1), :].T
        return out

    wq = pad_oc(np.asarray(q_w, np.float32)) * 8.0
    kv = np.asarray(kv_w, np.float32)
    wkv = np.concatenate([pad_oc(kv[:C]), pad_oc(kv[C:])], axis=1) * 8.0

    # [192,1,3,3] -> [256, 9, 128] diag, slots = DW_PAIRS order + center
    slot_tap = [0, 2, 3, 5, 6, 8, 1, 7, 4]

    def pad_dw(w):
        out = np.zeros((CP, 9, 128), np.float32)
        for h in range(HEADS):
            for j in range(CH):
                cp = CPH * h + j
                taps = w[CH * h + j, 0].reshape(9)
                for s, t in enumerate(slot_tap):
                    out[cp, s, cp % 128] = taps[t]
        return out * 32.0

    qdw = pad_dw(np.asarray(q_dw_w, np.float32))
    kvd = np.asarray(kv_dw_w, np.float32)
    kvdw = np.concatenate([pad_dw(kvd[:C]), pad_dw(kvd[C:])], axis=0)

    gamma_f = float(gamma)
    # wlin (bf16) = beta*lin padded; W_comb = attn x wlin -> tp = 256x t'_true
    lin = np.asarray(linear_w, np.float32) * float(beta)
    wlin = np.zeros((CP, C), np.float32)
    for h in range(HEADS):
        wlin[CPH * h:CPH * h + CH, :] = lin[:, CH * h:CH * (h + 1)].T
    wlin8 = wlin.reshape(2, 128, C).transpose(1, 0, 2)

    # t1c8 = gamma*t1; wf18 = ffn1_w^T * 8/gamma -> fp1 = 8x f1_true
    wf1 = np.asarray(ffn1_w, np.float32).T * (8.0 / gamma_f)
    wf18 = wf1.reshape(2, 96, 768).transpose(1, 0, 2)
    # wf28 = ffn2_w^T * 8*delta -> fp2 = 8*delta*f2_true
    wf2 = np.asarray(ffn2_w, np.float32).T * (8.0 * float(delta))
    wf28 = wf2.reshape(3, 2, 128, C).transpose(2, 0, 1, 3)
    wpr = np.asarray(proj_w, np.float32).T.reshape(2, 96, C).transpose(1, 0, 2)

    tempb = np.zeros((128, 2), np.float32)
    tv = np.asarray(temperature, np.float32).reshape(HEADS)
    for h in range(HEADS):
        tempb[64 * (h % 2):64 * (h % 2) + 64, h // 2] = tv[h]

    gscv = np.full((128, 1), gamma_f / 256.0, np.float32)
    id128 = np.eye(128, dtype=np.float32)

    return {
        "_yscale": gamma_f * float(alpha),
        "wq": wq.reshape(2, 96, CP).transpose(1, 0, 2).astype(f8),
        "wkv": wkv.reshape(2, 96, 2 * CP).transpose(1, 0, 2).astype(f8),
        "qdw": qdw.astype(f8), "kvdw": kvdw.astype(f8),
        "wlin": wlin8.astype(bf16).copy(), "wf1": wf18.astype(f8),
        "wf2": wf28.astype(f8), "wpr": wpr.astype(bf16).copy(),
        "tempb": tempb, "gamma": gscv,
        "id128": id128,
    }


def _make_in_maps(x, y, shared):
    shared = dict(shared)
    yscale = shared.pop("_yscale")
    in_maps = []
    for c in range(N_CORES):
        bi, s = c // 2, c % 2
        r0 = s * HLOC
        xe = np.zeros((C, ER, EC), np.float32)
        ye = np.zeros((C, ER, EC), np.float32)
        rlo, rhi = max(r0 - 1, 0), min(r0 + HLOC + 1, H)
        elo = rlo - (r0 - 1)
        xe[:, elo:elo + (rhi - rlo), 1:129] = x[bi, :, rlo:rhi, :]
        ye[:, elo:elo + (rhi - rlo), 1:129] = y[bi, :, rlo:rhi, :]
        m = dict(shared)
        m["xe"] = xe.reshape(2, 96, NEXT).transpose(1, 2, 0).astype(f8)
        m["ye"] = ye.reshape(2, 96, NEXT).transpose(1, 2, 0).astype(f8)
        m["yc"] = (yscale * y[bi, :, r0:r0 + HLOC, :]
                   ).reshape(C, NLOC).astype(bf16)
        in_maps.append(m)
    return in_maps


def kernel(**inputs):
    x = np.asarray(inputs["x"], np.float32)
    y = np.asarray(inputs["y"], np.float32)
    shared = _prep_weights(
        inputs["q_w"], inputs["q_dw_w"], inputs["kv_w"], inputs["kv_dw_w"],
        inputs["linear_w"], inputs["proj_w"], inputs["ffn1_w"], inputs["ffn2_w"],
        inputs["temperature"], inputs["alpha"], inputs["beta"],
        inputs["gamma"], inputs["delta"])

    in_maps = _make_in_maps(x, y, shared)

    nc = _get_nc()
    res = run_bass_kernel_spmd(nc, in_maps, list(range(N_CORES)))
    out = np.empty((B, C, H, W), np.float32)
    for c in range(N_CORES):
        bi, s = c // 2, c % 2
        out[bi, :, s * HLOC:(s + 1) * HLOC, :] = \
            res.results[c]["out"].reshape(C, HLOC, W)
    return out



# revision 76
# speedup vs baseline: 1.2071x; 1.0155x over previous
"""Trainium2 Bass kernel for nn_CDEM_62079457296798 (channel-attention
transformer block).

Sharding: 8 cores = 4 batches x 2 spatial halves (64 rows + 1 halo row each).
Cross-core communication: two small per-band-pair AllReduces carrying the
channel-attention Gram matrices and q/k l2-norm sums; everything else local.

Layout: channel-major activations [C_part, pixels_free]; attention channels
padded 48 -> 64 per head. Heavy use of fp8e4m3 DoubleRow matmuls (2 K-planes
per instruction; planes interleaved in memory so the PE streams 2 rows/cycle):
the q/kv 1x1 convs pair the 192 input channels as [96, 2]; the depthwise 3x3
runs as 4 tap-pair DoubleRow matmuls with diagonal [128, 2, 128] weights
(overlapping-stride pair APs over the padded image) + 1 bf16-free center tap;
ffn1/ffn2 pair K the same way. q/k sq-norms ride the Gram matmuls
(qg = q^T [q|k], kg = k^T k; diag extracted via masked reduce). z and lin are
fused: W_comb = beta*lin*attn is built once after softmax, so the per-chunk
trunk is W_comb^T v -> t1 -> ffn -> proj (proj in bf16). All runtime scalars
(alpha/beta/gamma/delta) are folded host-side; kernel-side rescales are
compile-time powers of two. The trunk is software-pipelined (tp one chunk
ahead) and interleaved with v-band production to keep the PE stream dense.
"""
import sys
sys.path.insert(0, '/opt/trn_rl_repo')

import numpy as np
import ml_dtypes

import bass_rust
from concourse import bacc, mybir, tile
from concourse.bass import _add_dep_helper
from concourse.bass_utils import run_bass_kernel_spmd

F32 = mybir.dt.float32
F32R = mybir.dt.float32r
BF16 = mybir.dt.bfloat16
FP8 = mybir.dt.float8e4
DRM = mybir.MatmulPerfMode.DoubleRow
AF = mybir.ActivationFunctionType
OP = mybir.AluOpType
bf16 = ml_dtypes.bfloat16
f8 = ml_dtypes.float8_e4m3fn

# depthwise 3x3 as 4 fp8 DoubleRow pairs + 1 single (tap index t = 3*(dr+1)+(dc+1));
# pair strides in elements of the [ER, EC] image (2 = two cols, 260 = two rows)
DW_PAIRS = [(0, 2, 2), (3, 5, 2), (6, 8, 2), (1, 7, 2 * 130)]
DW_SINGLE = 4


def _pair_ap(base, stride):
    raw = base.ap.copy()
    return bass_rust.AP(base.tensor, base.offset,
                        [raw[0], [stride, 2]] + list(raw[1:]))

N_CORES = 8
B, C, H, W = 4, 192, 128, 128
HEADS, CH = 4, 48
CPH = 64                # padded channels per head
CP = HEADS * CPH        # 256 padded attn channels
HLOC = 64               # image rows per core
ER, EC = 66, 130        # ext rows/cols (halo + zero pad)
NEXT = ER * EC          # 8580
NLOC = HLOC * W         # 8192
NCK = 16                # output chunks (4 rows x 128 = 512 px)
CONV_CHUNKS = [(i * 512, 512) for i in range(16)] + [(16 * 512, NEXT - 16 * 512)]
GRP = 2048
CONV_GROUPS = [(i * GRP, GRP) for i in range(4)] + [(4 * GRP, NEXT - 4 * GRP)]
KB = [(0, 128), (128, 64)]          # 192-channel K bands

DIRECT_PSUM_OUT = False  # DMA cannot read PSUM on TRN2


import os
STAGE = int(os.environ.get("KSTAGE", "4"))
KSUB = int(os.environ.get("KSUB", "4"))


class _StageDone(Exception):
    pass


def build_nc():
    nc = bacc.Bacc("TRN2", target_bir_lowering=False, debug=False,
                   num_devices=N_CORES)

    d_xe = nc.dram_tensor("xe", [96, NEXT, 2], FP8, kind="ExternalInput")
    d_ye = nc.dram_tensor("ye", [96, NEXT, 2], FP8, kind="ExternalInput")
    d_yc = nc.dram_tensor("yc", [C, NLOC], BF16, kind="ExternalInput")
    d_wq = nc.dram_tensor("wq", [96, 2, CP], FP8, kind="ExternalInput")
    d_wkv = nc.dram_tensor("wkv", [96, 2, 2 * CP], FP8, kind="ExternalInput")
    d_qdw = nc.dram_tensor("qdw", [CP, 9, 128], FP8, kind="ExternalInput")
    d_kvdw = nc.dram_tensor("kvdw", [2 * CP, 9, 128], FP8, kind="ExternalInput")
    d_wlin = nc.dram_tensor("wlin", [128, 2, C], BF16, kind="ExternalInput")
    d_wf1 = nc.dram_tensor("wf1", [96, 2, 768], FP8, kind="ExternalInput")
    d_wf2 = nc.dram_tensor("wf2", [128, 3, 2, C], FP8, kind="ExternalInput")
    d_wpr = nc.dram_tensor("wpr", [96, 2, C], BF16, kind="ExternalInput")
    d_tempb = nc.dram_tensor("tempb", [128, 2], F32, kind="ExternalInput")
    d_gamma = nc.dram_tensor("gamma", [128, 1], F32, kind="ExternalInput")
    d_id128 = nc.dram_tensor("id128", [128, 128], F32, kind="ExternalInput")
    d_out = nc.dram_tensor("out", [C, NLOC], F32, kind="ExternalOutput")
    cc_in = nc.dram_tensor("cc_in", [112, 228], F32)
    cc_out = nc.dram_tensor("cc_out", [112, 228], F32)

    with tile.TileContext(nc) as tc:
        with (
            tc.tile_pool(name="sbw", bufs=1) as sbw,      # weights/consts
            tc.tile_pool(name="sbpre", bufs=2) as sbpre,  # conv1x1 out (ext img)
            tc.tile_pool(name="sbin", bufs=5) as sbin,    # streamed conv inputs
            tc.tile_pool(name="sbqk", bufs=4) as sbqk,    # q/k chunk tiles
            tc.tile_pool(name="sbT", bufs=1) as sbT,      # qT/kT/v persistents
            tc.tile_pool(name="sbs", bufs=1) as sbs,      # small attn tiles
            tc.tile_pool(name="sbc", bufs=3) as sbc,      # trunk chunk pipeline
            tc.tile_pool(name="sbg", bufs=6) as sbg,      # gelu chunk tiles
            tc.tile_pool(name="pcv", bufs=3, space="PSUM") as pcv,
            tc.tile_pool(name="pdw", bufs=2, space="PSUM") as pdw,
            tc.tile_pool(name="psm", bufs=1, space="PSUM") as psm,
        ):
            # ---------- weights ----------
            wq_t = sbw.tile([96, 2, CP], FP8, tag="wq", name="wq")
            wkv_t = sbw.tile([96, 2, 2 * CP], FP8, tag="wkv", name="wkv")
            nc.sync.dma_start(wq_t[:], d_wq.ap())
            qdw_t = [sbw.tile([128, 9, 128], FP8, tag=f"qdw{m}", name=f"qdw{m}") for m in range(2)]
            kvdw_t = [sbw.tile([128, 9, 128], FP8, tag=f"kvdw{m}", name=f"kvdw{m}") for m in range(4)]
            id128 = sbw.tile([128, 128], F32, tag="id128", name="id128")
            for m in range(2):
                nc.sync.dma_start(qdw_t[m][:], d_qdw[128 * m:128 * (m + 1)])
            nc.sync.dma_start(id128[:], d_id128.ap())

            def load_kv_weights():
                nc.sync.dma_start(wkv_t[:], d_wkv.ap())
                for m in range(4):
                    nc.sync.dma_start(kvdw_t[m][:], d_kvdw[128 * m:128 * (m + 1)])
            wlin_t = sbw.tile([128, 2, C], BF16, tag="wlin", name="wlin")
            wf1_t = sbw.tile([96, 2, 768], FP8, tag="wf1", name="wf1")
            wf2_t = sbw.tile([128, 3, 2, C], FP8, tag="wf2", name="wf2")
            wpr_t = sbw.tile([96, 2, C], BF16, tag="wpr", name="wpr")
            tempb = sbw.tile([128, 2], F32, tag="tempb", name="tempb")
            gscv = sbw.tile([128, 1], F32, tag="gscv", name="gscv")

            def load_trunk_weights():
                nc.sync.dma_start(wlin_t[:], d_wlin.ap())
                nc.sync.dma_start(wf1_t[:], d_wf1.ap())
                nc.sync.dma_start(wf2_t[:], d_wf2.ap())
                nc.sync.dma_start(wpr_t[:], d_wpr.ap())
                nc.sync.dma_start(tempb[:], d_tempb.ap())
                nc.sync.dma_start(gscv[:], d_gamma.ap())

            # persistent attn-path results; qkT packs q (cols 0:112) and
            # k (cols 112:224) transposed per band-pair
            qkT = [sbT.tile([128, 64, 224], BF16, tag=f"qkT{p}", name=f"qkT{p}")
                   for p in range(2)]
            vband = [sbT.tile([128, NLOC], BF16, tag=f"v{m}", name=f"v{m}") for m in range(2)]
            # gram + norm accumulators live in the tp-tag PSUM (idle pre-trunk)
            qgacc = pcv.tile([112, 448], F32, tag="tp", bufs=2, name="qgacc")
            kgacc = pcv.tile([112, 224], F32, tag="tp", bufs=2, name="kgacc")

            # ============ q/k/v production ============
            def band_producer(src_dram, w_t, dw_tiles, m, sink,
                              collect_mms=None):
                """One 128-wide band: conv1x1 (fp8 DR) + depthwise 3x3.
                Returns (need_groups, emit_dw) for interleaved emission."""
                pre = sbpre.tile([128, ER, EC], FP8, tag="pre", name="pre")
                pref = pre[:].rearrange("p a b -> p (a b)")
                state = {"g": 0, "ci": 0}

                def need_groups(ng):
                    while state["g"] < min(ng, len(CONV_GROUPS)):
                        g0, gn = CONV_GROUPS[state["g"]]
                        xc = sbin.tile([96, GRP, 2], FP8, tag="xin", name="xin")
                        nc.sync.dma_start(xc[:, :gn, :],
                                          src_dram[:, g0:g0 + gn, :])
                        for c0 in range(0, gn, 512):
                            cn = min(512, gn - c0)
                            ps = pcv.tile([128, 512], F32, tag="cv", name="cv")
                            mm = nc.tensor.matmul(
                                ps[:, :cn],
                                w_t[:, :, 128 * m:128 * (m + 1)],
                                xc[:, c0:c0 + cn, :].rearrange(
                                    "p n two -> p two n"),
                                start=True, stop=True, perf_mode=DRM)
                            if collect_mms is not None:
                                collect_mms.append(mm)
                            if state["ci"] % 2 == 0:
                                nc.vector.tensor_copy(
                                    pref[:, g0 + c0:g0 + c0 + cn], ps[:, :cn])
                            else:
                                nc.scalar.copy(
                                    pref[:, g0 + c0:g0 + c0 + cn], ps[:, :cn])
                            state["ci"] += 1
                        state["g"] += 1

                def emit_dw(ck):
                    r0 = 1 + 4 * ck
                    dp = pdw.tile([128, 4, 128], F32, tag="dw", name="dw")
                    for i, (ta, tb, stride) in enumerate(DW_PAIRS):
                        dra, dca = ta // 3 - 1, ta % 3 - 1
                        base = pre[:, r0 + dra:r0 + 4 + dra,
                                   1 + dca:129 + dca]
                        nc.tensor.matmul(
                            dp[:], dw_tiles[m][:, 2 * i:2 * i + 2, :],
                            _pair_ap(base, stride),
                            start=(i == 0), stop=False, perf_mode=DRM)
                    nc.tensor.matmul(
                        dp[:], dw_tiles[m][:, 8, :],
                        pre[:, r0:r0 + 4, 1:129],
                        start=False, stop=True)
                    sink(m, ck, dp[:].rearrange("p a b -> p (a b)"))

                return need_groups, emit_dw

            def conv_dw_path(src_dram, w_t, dw_tiles, n_mb, sink, m_off=0,
                             collect_mms=None):
                for m in range(m_off, m_off + n_mb):
                    ng, edw = band_producer(src_dram, w_t, dw_tiles, m, sink,
                                            collect_mms)
                    ng(len(CONV_GROUPS))
                    for ck in range(NCK if KSUB >= 2 else 0):
                        edw(ck)

            def qk_sink(coff):
                qcbig = [None]

                def sink(m, ck, flat):
                    j = ck % 4
                    if j == 0:
                        qcbig[0] = sbqk.tile([128, 2048], BF16, tag="qkc", name="qkc")
                    qc = qcbig[0][:, 512 * j:512 * (j + 1)]
                    if ck % 2 == 0:
                        nc.vector.tensor_copy(qc, flat)
                    else:
                        nc.scalar.copy(qc, flat)
                    if KSUB >= 4 and j == 3:
                        nc.scalar.dma_start_transpose(
                            qkT[m][:, 4 * ck - 12:4 * ck + 4, coff:coff + 112],
                            qcbig[0][0:112, :])
                return sink

            def v_sink(m, ck, flat):
                dst = vband[m - 2]
                if ck % 2 == 0:
                    nc.vector.tensor_copy(dst[:, ck * 512:(ck + 1) * 512], flat)
                else:
                    nc.scalar.copy(dst[:, ck * 512:(ck + 1) * 512], flat)

            sinkq = qk_sink(0)
            sinkk = qk_sink(112)
            sqv = sbs.tile([128, 2], F32, tag="sqv", name="sqv")
            skv = sbs.tile([128, 2], F32, tag="skv", name="skv")

            def run_band(src, w_t, dwt, m, sink):
                ng, edw = band_producer(src, w_t, dwt, m, sink)
                ng(len(CONV_GROUPS))
                for ck in range(NCK if KSUB >= 2 else 0):
                    edw(ck)

            def gram_chunks(p, ck0, ck1):
                # gram + q/k sq-norms for band-pair p (qg: [q^T q | q^T k],
                # kg: k^T k)
                for ck in range(ck0, ck1):
                    nc.tensor.matmul(qgacc[:, 224 * p:224 * (p + 1)],
                                     qkT[p][:, ck, 0:112], qkT[p][:, ck, :],
                                     start=(ck == 0), stop=(ck == 63))
                    nc.tensor.matmul(kgacc[:, 112 * p:112 * (p + 1)],
                                     qkT[p][:, ck, 112:224],
                                     qkT[p][:, ck, 112:224],
                                     start=(ck == 0), stop=(ck == 63))

            def finish_ar():
                gsb = sbs.tile([112, 228], F32, tag="gsb", name="gsb")
                for p in range(2):
                    dsc = sbs.tile([112, 112], F32, tag=f"dsc{p}", name=f"dsc{p}")
                    nc.vector.tensor_tensor(
                        out=dsc[:], in0=qgacc[:, 224 * p:224 * p + 112],
                        in1=id128[0:112, 0:112], op=OP.mult)
                    nc.vector.tensor_reduce(sqv[0:112, p:p + 1], dsc[:],
                                            axis=mybir.AxisListType.X, op=OP.add)
                    dsc2 = sbs.tile([112, 112], F32, tag=f"dsc2{p}", name=f"dsc2{p}")
                    nc.vector.tensor_tensor(
                        out=dsc2[:], in0=kgacc[:, 112 * p:112 * (p + 1)],
                        in1=id128[0:112, 0:112], op=OP.mult)
                    nc.vector.tensor_reduce(skv[0:112, p:p + 1], dsc2[:],
                                            axis=mybir.AxisListType.X, op=OP.add)
                    nc.vector.tensor_copy(gsb[:, 112 * p:112 * (p + 1)],
                                          qgacc[:, 224 * p + 112:224 * (p + 1)])
                nc.vector.tensor_copy(gsb[:, 224:226], sqv[0:112, :])
                nc.vector.tensor_copy(gsb[:, 226:228], skv[0:112, :])
                nc.scalar.dma_start(cc_in.ap()[:, :], gsb[:, :])
                nc.gpsimd.collective_compute(
                    "AllReduce", OP.add,
                    replica_groups=[[0, 1], [2, 3], [4, 5], [6, 7]],
                    ins=[cc_in.ap()], outs=[cc_out.ap()])

            run_band(d_xe, wq_t, qdw_t, 0, sinkq)
            load_kv_weights()
            run_band(d_ye, wkv_t, kvdw_t, 0, sinkk)
            # q band 1 with gram p0 interleaved (needs only q0/k0)
            ngq1, edwq1 = band_producer(d_xe, wq_t, qdw_t, 1, sinkq)
            ngq1(len(CONV_GROUPS))
            for ck in range(NCK if KSUB >= 2 else 0):
                edwq1(ck)
                if STAGE >= 3:
                    gram_chunks(0, 4 * ck, 4 * ck + 4)
            # k band 1 with gram p1 lag-interleaved (4 chunks behind the
            # transposes feeding qkT[1])
            ngk1, edwk1 = band_producer(d_ye, wkv_t, kvdw_t, 1, sinkk)
            ngk1(len(CONV_GROUPS))
            for ck in range(NCK if KSUB >= 2 else 0):
                edwk1(ck)
                if STAGE >= 3 and ck >= 4:
                    gram_chunks(1, 4 * (ck - 4), 4 * (ck - 4) + 4)
            load_trunk_weights()

            if STAGE < 3:
                oc0 = sbs.tile([128, 2], F32, tag="oc0d", name="oc0d")
                nc.vector.tensor_copy(oc0[:], sqv[:])
                nc.sync.dma_start(d_out[0:128, 0:2], oc0[:])
            if STAGE >= 3:
                # prefetch ALL of v2's conv inputs, then dispatch the single
                # AllReduce: v2's DMA-free dw work covers the wire phase,
                # which blocks input DMAs for ~15us
                v_mms = []
                ngv2, edwv2 = band_producer(d_ye, wkv_t, kvdw_t, 2, v_sink,
                                            collect_mms=v_mms)
                ngv2(len(CONV_GROUPS))
                gram_chunks(1, 48, 64)
                finish_ar()
                for ck in range(NCK):
                    edwv2(ck)
                vng3, vdw3 = band_producer(d_ye, wkv_t, kvdw_t, 3, v_sink,
                                           collect_mms=v_mms)
                vng3(len(CONV_GROUPS))
                for ck in range(6):
                    vdw3(ck)

                gg = sbs.tile([112, 224], F32, tag="gg", name="gg")
                sqg = sbs.tile([128, 2], F32, tag="sqg", name="sqg")
                skg = sbs.tile([128, 2], F32, tag="skg", name="skg")
                nc.vector.memset(sqg[:], 1.0)
                nc.vector.memset(skg[:], 1.0)
                nc.sync.dma_start(gg[:], cc_out.ap()[:, 0:224])
                nc.sync.dma_start(sqg[0:112, :], cc_out.ap()[:, 224:226])
                nc.sync.dma_start(skg[0:112, :], cc_out.ap()[:, 226:228])

                # ============ attention finalize ============
                def rsqrt_newton(tag, s_t):
                    sc = sbs.tile([128, 2], F32, tag=tag + "_c")
                    nc.vector.tensor_scalar_max(sc[:], s_t[:], 1e-24)
                    rt = sbs.tile([128, 2], F32, tag=tag + "_s")
                    nc.scalar.activation(rt[:], sc[:], AF.Sqrt)
                    r0 = sbs.tile([128, 2], F32, tag=tag + "_r0")
                    nc.vector.reciprocal(r0[:], rt[:])
                    rr = sbs.tile([128, 2], F32, tag=tag + "_rr")
                    nc.vector.tensor_tensor(out=rr[:], in0=r0[:], in1=r0[:], op=OP.mult)
                    t1_ = sbs.tile([128, 2], F32, tag=tag + "_t1")
                    nc.vector.scalar_tensor_tensor(out=t1_[:], in0=sc[:], scalar=-0.5,
                                                   in1=rr[:], op0=OP.mult, op1=OP.mult)
                    nc.vector.tensor_scalar_add(t1_[:], t1_[:], 1.5)
                    rv = sbs.tile([128, 2], F32, tag=tag)
                    nc.vector.tensor_tensor(out=rv[:], in0=r0[:], in1=t1_[:], op=OP.mult)
                    return rv

                rq = rsqrt_newton("rq", sqg)
                rk = rsqrt_newton("rk", skg)
                srow = sbs.tile([128, 2], F32, tag="srow", name="srow")
                nc.vector.tensor_tensor(out=srow[:], in0=rq[:], in1=tempb[:], op=OP.mult)

                srow_r, scol_r = [], []
                for p in range(2):
                    for src, lst, nm in ((srow, srow_r, "sr"), (rk, scol_r, "sc")):
                        fp = psm.tile([1, 112], F32, tag="sm", name="sm")
                        nc.tensor.transpose(fp[:], src[0:112, p:p + 1],
                                            id128[0:112, 0:112])
                        fr = sbs.tile([1, 112], F32R, tag=f"{nm}{p}", name=f"{nm}{p}")
                        nc.vector.tensor_copy(fr[:], fp[:])
                        lst.append(fr)

                # W_comb = beta*lin*attn, [v-ch(pad 128), plane p, out 192];
                # pad v rows stay zero (vband pad rows are zero anyway)
                wcs = sbs.tile([128, 2, C], BF16, tag="wcs", name="wcs")
                nc.gpsimd.memset(wcs[:], 0.0)
                for p in range(2):
                    spair = psm.tile([112, 112], F32, tag="sm", name="sm")
                    nc.tensor.matmul(spair[:], srow_r[p][:], scol_r[p][:],
                                     start=True, stop=True)
                    lg = sbs.tile([112, 112], F32, tag="lg", name="lg")
                    nc.vector.tensor_tensor(out=lg[:], in0=gg[:, 112 * p:112 * (p + 1)],
                                            in1=spair[:], op=OP.mult)
                    at16 = sbs.tile([112, 112], BF16, tag="at16", name="at16")
                    for e in range(2):
                        sl = slice(64 * e, 64 * e + 48)
                        mx = sbs.tile([112, 1], F32, tag="mx", name="mx")
                        nc.vector.tensor_reduce(mx[sl, :], lg[sl, sl],
                                                axis=mybir.AxisListType.X, op=OP.max)
                        exh = sbs.tile([112, 112], F32, tag="exh", name="exh")
                        nc.vector.tensor_scalar(out=exh[sl, 0:48], in0=lg[sl, sl],
                                                scalar1=mx[sl, :], scalar2=None,
                                                op0=OP.subtract)
                        ex2 = sbs.tile([112, 112], F32, tag="ex2", name="ex2")
                        den = sbs.tile([112, 1], F32, tag="den", name="den")
                        nc.scalar.activation(ex2[sl, 0:48], exh[sl, 0:48], AF.Exp,
                                             accum_out=den[sl, :])
                        rc0 = sbs.tile([112, 1], F32, tag="rc0", name="rc0")
                        nc.vector.reciprocal(rc0[sl, :], den[sl, :])
                        nt = sbs.tile([112, 1], F32, tag="nt", name="nt")
                        nc.vector.tensor_tensor(out=nt[sl, :], in0=den[sl, :],
                                                in1=rc0[sl, :], op=OP.mult)
                        nc.vector.tensor_scalar(out=nt[sl, :], in0=nt[sl, :],
                                                scalar1=-1.0, scalar2=2.0,
                                                op0=OP.mult, op1=OP.add)
                        rc1 = sbs.tile([112, 1], F32, tag="rc1", name="rc1")
                        nc.vector.tensor_tensor(out=rc1[sl, :], in0=rc0[sl, :],
                                                in1=nt[sl, :], op=OP.mult)
                        nc.vector.tensor_scalar(out=at16[sl, 0:48], in0=ex2[sl, 0:48],
                                                scalar1=rc1[sl, :], scalar2=None,
                                                op0=OP.mult)
                        wcp = pcv.tile([128, 512], F32, tag="cv", name="cv")
                        nc.tensor.matmul(wcp[sl, :C], at16[sl, 0:48],
                                         wlin_t[64 * e:64 * e + 48, p, :],
                                         start=True, stop=True)
                        nc.vector.tensor_copy(wcs[sl, p, :], wcp[sl, :C])

                if STAGE < 4:
                    for ck in range(6, NCK):
                        vdw3(ck)
                    ocx = sbs.tile([112, 64], F32, tag="ocx", name="ocx")
                    nc.vector.tensor_copy(ocx[:], wcs[0:112, 0, 0:64])
                    nc.sync.dma_start(d_out[0:112, 0:64], ocx[:])
                # ==== software-pipelined trunk, interleaved with v band-3 ====
                # stage A(ck): v-dw chunk (6 ahead) + tp matmuls — one iter ahead
                # stage B(ck): t1 -> ffn1 -> gelu -> ffn2 -> t2 -> proj
                tp_q, t1_q = {}, {}

                def stage_a(ck):
                    if ck + 6 < NCK:
                        vdw3(ck + 6)
                    c0 = ck * 512
                    # fused z+lin: tp = W_comb^T v = 256x t'_true
                    tp = [pcv.tile([128, 512], F32, tag="tp", bufs=2,
                                   name="tp") for _ in range(2)]
                    for mi in range(2):
                        for p in range(2):
                            nc.tensor.matmul(tp[mi][:96, :],
                                             wcs[:, p, 96 * mi:96 * (mi + 1)],
                                             vband[p][:, c0:c0 + 512],
                                             start=(p == 0), stop=(p == 1))
                    tp_q[ck] = tp

                def stage_t1(ck):
                    tp = tp_q.pop(ck)
                    c0 = ck * 512
                    ycn = sbc.tile([96, 2, 512], BF16, tag="ycn", name="ycn")
                    for mi in range(2):
                        nc.sync.dma_start(ycn[:, mi, :],
                                          d_yc[96 * mi:96 * (mi + 1), c0:c0 + 512])
                    # t1f = gamma*t1 = ycn(= g*a*y) + tp * (gamma/256)
                    t1f = sbc.tile([96, 2, 512], BF16, tag="t1f", name="t1f")
                    for mi in range(2):
                        nc.vector.scalar_tensor_tensor(
                            out=t1f[:, mi, :], in0=tp[mi][:96, :],
                            scalar=gscv[0:96, :],
                            in1=ycn[:, mi, :], op0=OP.mult, op1=OP.add)
                    t1c8 = sbc.tile([96, 512, 2], FP8, tag="t1c8", name="t1c8")
                    nc.vector.tensor_copy(
                        t1c8[:].rearrange("p n two -> p two n"), t1f[:])
                    t1_q[ck] = (t1f, t1c8)

                def stage_b(ck):
                    t1f, t1c8 = t1_q.pop(ck)
                    c0 = ck * 512
                    # ffn1 + gelu: fp1 = 8x f1_true; gc8 = gelu(f1_true)
                    gc8 = [sbg.tile([128, 512, 2], FP8, tag="gc8", name="gc8")
                           for _ in range(3)]
                    for mt in range(6):
                        fp1 = pcv.tile([128, 512], F32, tag="cv", name="cv")
                        nc.tensor.matmul(fp1[:],
                                         wf1_t[:, :, 128 * mt:128 * (mt + 1)],
                                         t1c8[:].rearrange("p n two -> p two n"),
                                         start=True, stop=True,
                                         perf_mode=DRM)
                        nc.scalar.activation(gc8[mt // 2][:, :, mt % 2], fp1[:],
                                             AF.Gelu, scale=0.125)
                    # ffn2: fp2 = 8*delta*f2_true; t2 = t1f + fp2/8
                    t2c = sbc.tile([96, 2, 512], BF16, tag="t2c", name="t2c")
                    for mi in range(2):
                        fp2 = pcv.tile([128, 512], F32, tag="cv", name="cv")
                        for jp in range(3):
                            nc.tensor.matmul(fp2[:96, :],
                                             wf2_t[:, jp, :, 96 * mi:96 * (mi + 1)],
                                             gc8[jp][:].rearrange(
                                                 "p n two -> p two n"),
                                             start=(jp == 0),
                                             stop=(jp == 2), perf_mode=DRM)
                        nc.vector.scalar_tensor_tensor(
                            out=t2c[:, mi, :], in0=fp2[:96, :], scalar=0.125,
                            in1=t1f[:, mi, :], op0=OP.mult, op1=OP.add)
                    # proj (bf16): accumulate over the two 96-ch planes
                    for mi, (mo, ms) in enumerate(KB):
                        pp = pcv.tile([128, 512], F32, tag="cv", name="cv")
                        for pl in range(2):
                            nc.tensor.matmul(pp[:ms, :],
                                             wpr_t[:, pl, mo:mo + ms],
                                             t2c[:, pl, :],
                                             start=(pl == 0), stop=(pl == 1))
                        oc = sbc.tile([128, 512], F32, tag=f"oc{mi}", name=f"oc{mi}")
                        if mi == 0:
                            nc.scalar.copy(oc[:ms, :], pp[:ms, :])
                        else:
                            nc.vector.tensor_copy(oc[:ms, :], pp[:ms, :])
                        nc.sync.dma_start(d_out[mo:mo + ms, c0:c0 + 512],
                                          oc[:ms, :])

                if STAGE >= 4:
                    stage_a(0)
                    stage_t1(0)
                    for ck in range(NCK):
                        if ck + 1 < NCK:
                            stage_a(ck + 1)
                        stage_b(ck)
                        if ck + 1 < NCK:
                            stage_t1(ck + 1)

    nc.compile()
    return nc


_NC = None


def _get_nc():
    global _NC
    if _NC is None:
        _NC = build_nc()
    return _NC


def _prep_weights(q_w, q_dw_w, kv_w, kv_dw_w, linear_w, proj_w, ffn1_w, ffn2_w,
                  temperature, alpha, beta, gamma, delta):
    def pad_oc(w):  # [192 real oc, ic] -> [ic, 256 padded oc]
        out = np.zeros((C, CP), np.float32)
        for h in range(HEADS):
            out[:, CPH * h:CPH * h + CH] = w[CH * h:CH * (h + 1), :].T
        return out

    wq = pad_oc(np.asarray(q_w, np.float32)) * 8.0
    kv = np.asarray(kv_w, np.float32)
    wkv = np.concatenate([pad_oc(kv[:C]), pad_oc(kv[C:])], axis=1) * 8.0

    # [192,1,3,3] -> [256, 9, 128] diag, slots = DW_PAIRS order + center
    slot_tap = [0, 2, 3, 5, 6, 8, 1, 7, 4]

    def pad_dw(w):
        out = np.zeros((CP, 9, 128), np.float32)
        for h in range(HEADS):
            for j in range(CH):
                cp = CPH * h + j
                taps = w[CH * h + j, 0].reshape(9)
                for s, t in enumerate(slot_tap):
                    out[cp, s, cp % 128] = taps[t]
        return out * 32.0

    qdw = pad_dw(np.asarray(q_dw_w, np.float32))
    kvd = np.asarray(kv_dw_w, np.float32)
    kvdw = np.concatenate([pad_dw(kvd[:C]), pad_dw(kvd[C:])], axis=0)

    gamma_f = float(gamma)
    # wlin (bf16) = beta*lin padded; W_comb = attn x wlin -> tp = 256x t'_true
    lin = np.asarray(linear_w, np.float32) * float(beta)
    wlin = np.zeros((CP, C), np.float32)
    for h in range(HEADS):
        wlin[CPH * h:CPH * h + CH, :] = lin[:, CH * h:CH * (h + 1)].T
    wlin8 = wlin.reshape(2, 128, C).transpose(1, 0, 2)

    # t1c8 = gamma*t1; wf18 = ffn1_w^T * 8/gamma -> fp1 = 8x f1_true
    wf1 = np.asarray(ffn1_w, np.float32).T * (8.0 / gamma_f)
    wf18 = wf1.reshape(2, 96, 768).transpose(1, 0, 2)
    # wf28 = ffn2_w^T * 8*delta -> fp2 = 8*delta*f2_true
    wf2 = np.asarray(ffn2_w, np.float32).T * (8.0 * float(delta))
    wf28 = wf2.reshape(3, 2, 128, C).transpose(2, 0, 1, 3)
    wpr = np.asarray(proj_w, np.float32).T.reshape(2, 96, C).transpose(1, 0, 2)

    tempb = np.zeros((128, 2), np.float32)
    tv = np.asarray(temperature, np.float32).reshape(HEADS)
    for h in range(HEADS):
        tempb[64 * (h % 2):64 * (h % 2) + 64, h // 2] = tv[h]

    gscv = np.full((128, 1), gamma_f / 256.0, np.float32)
    id128 = np.eye(128, dtype=np.float32)

    return {
        "_yscale": gamma_f * float(alpha),
        "wq": wq.reshape(2, 96, CP).transpose(1, 0, 2).astype(f8),
        "wkv": wkv.reshape(2, 96, 2 * CP).transpose(1, 0, 2).astype(f8),
        "qdw": qdw.astype(f8), "kvdw": kvdw.astype(f8),
        "wlin": wlin8.astype(bf16).copy(), "wf1": wf18.astype(f8),
        "wf2": wf28.astype(f8), "wpr": wpr.astype(bf16).copy(),
        "tempb": tempb, "gamma": gscv,
        "id128": id128,
    }


def _make_in_maps(x, y, shared):
    shared = dict(shared)
    yscale = shared.pop("_yscale")
    in_maps = []
    for c in range(N_CORES):
        bi, s = c // 2, c % 2
        r0 = s * HLOC
        xe = np.zeros((C, ER, EC), np.float32)
        ye = np.zeros((C, ER, EC), np.float32)
        rlo, rhi = max(r0 - 1, 0), min(r0 + HLOC + 1, H)
        elo = rlo - (r0 - 1)
        xe[:, elo:elo + (rhi - rlo), 1:129] = x[bi, :, rlo:rhi, :]
        ye[:, elo:elo + (rhi - rlo), 1:129] = y[bi, :, rlo:rhi, :]
        m = dict(shared)
        m["xe"] = xe.reshape(2, 96, NEXT).transpose(1, 2, 0).astype(f8)
        m["ye"] = ye.reshape(2, 96, NEXT).transpose(1, 2, 0).astype(f8)
        m["yc"] = (yscale * y[bi, :, r0:r0 + HLOC, :]
                   ).reshape(C, NLOC).astype(bf16)
        in_maps.append(m)
    return in_maps


def kernel(**inputs):
    x = np.asarray(inputs["x"], np.float32)
    y = np.asarray(inputs["y"], np.float32)
    shared = _prep_weights(
        inputs["q_w"], inputs["q_dw_w"], inputs["kv_w"], inputs["kv_dw_w"],
        inputs["linear_w"], inputs["proj_w"], inputs["ffn1_w"], inputs["ffn2_w"],
        inputs["temperature"], inputs["alpha"], inputs["beta"],
        inputs["gamma"], inputs["delta"])

    in_maps = _make_in_maps(x, y, shared)

    nc = _get_nc()
    res = run_bass_kernel_spmd(nc, in_maps, list(range(N_CORES)))
    out = np.empty((B, C, H, W), np.float32)
    for c in range(N_CORES):
        bi, s = c // 2, c % 2
        out[bi, :, s * HLOC:(s + 1) * HLOC, :] = \
            res.results[c]["out"].reshape(C, HLOC, W)
    return out

